# revision 1
# baseline (speedup 1.0000x reference)
"""PointNetLK on 8 TRN2 NeuronCores — batch-parallel, 2 samples/core.

Two device programs:
  prog1: 7 PointNet feature evals (tf + 6 finite-diff Jacobian evals)
  prog2: 10 LK iterations fully on-device (feat eval, pose solve via
         precomputed -pinv, exp_se3 Taylor, SE3 state update)
Host: means, J transforms, J/H/pinv solve, final 4x4 assembly.
"""

import numpy as np

B, N, NC, SPC = 16, 1024, 8, 2
MAXITER = 10

_BUILT = {}
TRACE = False
LAST_NS = 0


def _exp_se3_np(x):
    x = np.asarray(x, np.float64)
    w, v = x[..., :3], x[..., 3:]
    t2 = (w * w).sum(-1)
    t = np.sqrt(np.maximum(t2, 1e-300))
    small = t2 < 1e-12
    A = np.where(small, 1.0 - t2 / 6.0, np.sin(t) / t)
    Bc = np.where(small, 0.5 - t2 / 24.0, (1.0 - np.cos(t)) / np.maximum(t2, 1e-300))
    C = np.where(small, 1.0 / 6.0 - t2 / 120.0, (t - np.sin(t)) / np.maximum(t2 * t, 1e-300))
    z = np.zeros_like(t2)
    wx, wy, wz = w[..., 0], w[..., 1], w[..., 2]
    W = np.stack([
        np.stack([z, -wz, wy], -1),
        np.stack([wz, z, -wx], -1),
        np.stack([-wy, wx, z], -1)], -2)
    W2 = W @ W
    I = np.eye(3)
    R = I + A[..., None, None] * W + Bc[..., None, None] * W2
    V = I + Bc[..., None, None] * W + C[..., None, None] * W2
    tv = np.einsum('...ij,...j->...i', V, v)
    out = np.zeros(x.shape[:-1] + (4, 4))
    out[..., :3, :3] = R
    out[..., :3, 3] = tv
    out[..., 3, 3] = 1.0
    return out


def _feat_eval(nc, tc, bigps, sb, ts, l1t_ap, beff_ap, w2, w3, w4a, w4b, w5,
               x1, x2, x3, x4a, x4b, scr, scr2, fdst):
    import concourse.mybir as mybir
    Relu = mybir.ActivationFunctionType.Relu
    Copy = mybir.ActivationFunctionType.Copy
    mx = mybir.AluOpType.max
    F32 = mybir.dt.float32
    H = 512

    def mm_act(lhsT, rhs_tile, out_tile, bias):
        for h in range(2):
            p = bigps()
            nc.tensor.matmul(p[:, 0:H], lhsT, rhs_tile[:, h * H:(h + 1) * H],
                             start=True, stop=True)
            nc.scalar.activation(out_tile[:, h * H:(h + 1) * H], p[:, 0:H],
                                 Relu, bias=bias)

    mm_act(l1t_ap, ts, x1, beff_ap)
    mm_act(w2[:], x1, x2, 0.0)
    mm_act(w3[:], x2, x3, 0.0)
    mm_act(w4a[:], x3, x4a, 0.0)
    mm_act(w4b[:], x3, x4b, 0.0)
    for s, x4 in ((0, x4a), (1, x4b)):
        for j in range(8):
            pa = bigps()
            nc.tensor.matmul(pa[:, 0:H], w5[:, 128 * j:128 * (j + 1)],
                             x4[:, 0:H], start=True, stop=True)
            pb = bigps()
            nc.tensor.matmul(pb[:, 0:H], w5[:, 128 * j:128 * (j + 1)],
                             x4[:, H:2 * H], start=True, stop=True)
            nc.scalar.activation(scr2[:], pa[:, 0:H], Copy)
            nc.vector.scalar_tensor_tensor(
                out=scr[:], in0=scr2[:], scalar=0.0, in1=pb[:, 0:H],
                op0=mx, op1=mx)
            nc.vector.tensor_reduce(fdst[:, 8 * s + j:8 * s + j + 1], scr[:],
                                    axis=mybir.AxisListType.X, op=mx)


def _build_common(nc, tc, sb):
    import concourse.mybir as mybir
    F32 = mybir.dt.float32
    ts = sb.tile([6, 1024], F32)
    w2 = sb.tile([128, 128], F32)
    w3 = sb.tile([128, 128], F32)
    w4a = sb.tile([128, 128], F32)
    w4b = sb.tile([128, 128], F32)
    w5 = sb.tile([128, 1024], F32)
    x1 = sb.tile([128, 1024], F32)
    x2 = sb.tile([128, 1024], F32)
    x3 = sb.tile([128, 1024], F32)
    x4a = sb.tile([128, 1024], F32)
    x4b = sb.tile([128, 1024], F32)
    scr = sb.tile([128, 512], F32)
    scr2 = sb.tile([128, 512], F32)
    return ts, w2, w3, w4a, w4b, w5, x1, x2, x3, x4a, x4b, scr, scr2


def _build_prog1(n_evals=7):
    import concourse.bacc as bacc
    import concourse.mybir as mybir
    import concourse.tile as tile
    F32 = mybir.dt.float32
    nc = bacc.Bacc()
    d = {}
    for name, shp in (("TS", [6, 1024]), ("L1T", [6, 896]), ("BEFF", [128, 7]),
                      ("W2B", [128, 128]), ("W3B", [128, 128]),
                      ("W4A", [128, 128]), ("W4B", [128, 128]),
                      ("W5", [128, 1024])):
        d[name] = nc.declare_dram_parameter(name, shp, F32, isOutput=False)
    F7 = nc.declare_dram_parameter("F7", [128, 112], F32, isOutput=True)

    with tile.TileContext(nc) as tc:
        with (tc.tile_pool(name="sb", bufs=1) as sb,
              tc.tile_pool(name="psb", bufs=4, space="PSUM") as psb):
            ts, w2, w3, w4a, w4b, w5, x1, x2, x3, x4a, x4b, scr, scr2 = _build_common(nc, tc, sb)
            l1t = sb.tile([6, 896], F32)
            beff = sb.tile([128, 7], F32)
            feats = sb.tile([128, 112], F32)
            for t_, d_ in ((ts, d["TS"]), (l1t, d["L1T"]), (beff, d["BEFF"]),
                           (w2, d["W2B"]), (w3, d["W3B"]), (w4a, d["W4A"]),
                           (w4b, d["W4B"]), (w5, d["W5"])):
                nc.sync.dma_start(t_[:], d_[:])

            def bigps():
                return psb.tile([128, 512], F32, name="bp", tag="bp")

            for e in range(n_evals):
                _feat_eval(nc, tc, bigps, sb, ts, l1t[:, 128 * e:128 * e + 128],
                           beff[:, e:e + 1], w2, w3, w4a, w4b, w5,
                           x1, x2, x3, x4a, x4b, scr, scr2,
                           feats[:, 16 * e:16 * e + 16])
            nc.sync.dma_start(F7[:], feats[:])
    nc.finalize()
    return nc


def _build_prog2():
    import concourse.bacc as bacc
    import concourse.mybir as mybir
    import concourse.tile as tile
    F32 = mybir.dt.float32
    mul = mybir.AluOpType.mult
    add = mybir.AluOpType.add
    sub = mybir.AluOpType.subtract
    Copy = mybir.ActivationFunctionType.Copy
    nc = bacc.Bacc()
    d = {}
    for name, shp in (("TS", [6, 1024]), ("W1", [3, 64]), ("M1", [3, 2]),
                      ("PV", [128, 96]), ("TF", [128, 16]), ("EYE", [3, 3]),
                      ("W2B", [128, 128]), ("W3B", [128, 128]),
                      ("W4A", [128, 128]), ("W4B", [128, 128]),
                      ("W5", [128, 1024])):
        d[name] = nc.declare_dram_parameter(name, shp, F32, isOutput=False)
    O = nc.declare_dram_parameter("O", [6, 4], F32, isOutput=True)

    with tile.TileContext(nc) as tc:
        with (tc.tile_pool(name="sb", bufs=1) as sb,
              tc.tile_pool(name="psb", bufs=4, space="PSUM") as psb,
              tc.tile_pool(name="pss", bufs=4, space="PSUM") as pss):
            ts, w2, w3, w4a, w4b, w5, x1, x2, x3, x4a, x4b, scr, scr2 = _build_common(nc, tc, sb)
            w1 = sb.tile([3, 64], F32)
            m1 = sb.tile([3, 2], F32)
            pv = sb.tile([128, 96], F32)
            tf = sb.tile([128, 16], F32)
            feats = sb.tile([128, 16], F32)
            rr = sb.tile([128, 16], F32)
            l1t = sb.tile([6, 128], F32)
            beff2 = sb.tile([128, 1], F32)
            stg = sb.tile([64, 64], F32)
            prow = sb.tile([1, 64], F32)
            ones13 = sb.tile([1, 3], F32)
            eye = sb.tile([3, 3], F32)
            bc = sb.tile([3, 16], F32)
            Wm = sb.tile([3, 32], F32)
            W2s = sb.tile([3, 32], F32)
            t2I = sb.tile([3, 32], F32)
            u1 = sb.tile([3, 32], F32)
            u2 = sb.tile([3, 32], F32)
            Rg = sb.tile([3, 32], F32)
            RgT = sb.tile([3, 32], F32)
            VT = sb.tile([3, 32], F32)
            teff = sb.tile([3, 2], F32)
            Ra = sb.tile([3, 3], F32)
            Rb = sb.tile([3, 3], F32)
            RTa = sb.tile([3, 3], F32)
            RTb = sb.tile([3, 3], F32)
            ta = sb.tile([3, 1], F32)
            tb = sb.tile([3, 1], F32)

            for t_, d_ in ((ts, d["TS"]), (w1, d["W1"]), (m1, d["M1"]),
                           (pv, d["PV"]), (tf, d["TF"]), (eye, d["EYE"]),
                           (w2, d["W2B"]), (w3, d["W3B"]), (w4a, d["W4A"]),
                           (w4b, d["W4B"]), (w5, d["W5"])):
                nc.sync.dma_start(t_[:], d_[:])

            nc.vector.memset(l1t[:], 0.0)
            nc.vector.memset(prow[:], 0.0)
            nc.vector.memset(ones13[:], 1.0)
            nc.vector.memset(ta[:], 0.0)
            nc.vector.memset(tb[:], 0.0)
            nc.vector.tensor_copy(Ra[:], eye[:])
            nc.vector.tensor_copy(Rb[:], eye[:])
            nc.vector.tensor_copy(RTa[:], eye[:])
            nc.vector.tensor_copy(RTb[:], eye[:])

            def sps():
                return pss.tile([64, 512], F32, name="sp", tag="sp")

            def bigps():
                return psb.tile([128, 512], F32, name="bp", tag="bp")

            state = [(Ra, RTa, ta), (Rb, RTb, tb)]

            for it in range(MAXITER):
                # fold est_T and mean-shift into L1 weights/bias
                for s in range(SPC):
                    R, RT, t = state[s]
                    p = sps()
                    nc.tensor.matmul(p[0:3, 0:64], R[:], w1[:], start=True, stop=True)
                    if s == 0:
                        nc.scalar.activation(l1t[0:3, 0:64], p[0:3, 0:64], Copy)
                    else:
                        nc.scalar.activation(stg[0:3, 0:64], p[0:3, 0:64], Copy)
                        nc.sync.dma_start(l1t[3:6, 64:128], stg[0:3, 0:64])
                    p2 = sps()
                    nc.tensor.matmul(p2[0:3, 0:1], RT[:], m1[:, s:s + 1], start=True, stop=True)
                    nc.vector.tensor_tensor(out=teff[:, s:s + 1], in0=t[:], in1=p2[0:3, 0:1], op=sub)
                    p3 = sps()
                    nc.tensor.matmul(p3[0:64, 0:1], w1[:], teff[:, s:s + 1], start=True, stop=True)
                    if s == 0:
                        nc.scalar.activation(beff2[0:64, 0:1], p3[0:64, 0:1], Copy)
                    else:
                        nc.scalar.activation(stg[0:64, 32:33], p3[0:64, 0:1], Copy)
                        nc.sync.dma_start(beff2[64:128, 0:1], stg[0:64, 32:33])

                _feat_eval(nc, tc, bigps, sb, ts, l1t[:], beff2[:, 0:1],
                           w2, w3, w4a, w4b, w5, x1, x2, x3, x4a, x4b, scr, scr2,
                           feats[:, 0:16])
                nc.vector.tensor_tensor(out=rr[:], in0=feats[:], in1=tf[:], op=sub)

                for s in range(SPC):
                    R, RT, t = state[s]
                    o = 32 * s
                    pp = sps()
                    for j in range(8):
                        nc.tensor.matmul(pp[0:1, 0:6], rr[:, 8 * s + j:8 * s + j + 1],
                                         pv[:, 48 * s + 6 * j:48 * s + 6 * j + 6],
                                         start=(j == 0), stop=(j == 7))
                    nc.vector.tensor_copy(prow[0:1, o:o + 6], pp[0:1, 0:6])
                    # t2 = |w|^2 at col o+11
                    nc.vector.tensor_tensor(out=prow[0:1, o + 8:o + 11],
                                            in0=prow[0:1, o:o + 3],
                                            in1=prow[0:1, o:o + 3], op=mul)
                    nc.vector.tensor_reduce(prow[0:1, o + 11:o + 12],
                                            prow[0:1, o + 8:o + 11],
                                            axis=mybir.AxisListType.X, op=add)
                    t2 = prow[0:1, o + 11:o + 12]
                    # Taylor A,B,C at cols o+12..o+14 (Horner in-place)
                    for col, (c3, c2, c1, c0) in (
                            (12, (-1.0 / 5040, 1.0 / 120, -1.0 / 6, 1.0)),
                            (13, (-1.0 / 40320, 1.0 / 720, -1.0 / 24, 0.5)),
                            (14, (-1.0 / 362880, 1.0 / 5040, -1.0 / 120, 1.0 / 6))):
                        dst = prow[0:1, o + col:o + col + 1]
                        nc.vector.tensor_scalar(out=dst, in0=t2, scalar1=c3,
                                                scalar2=c2, op0=mul, op1=add)
                        nc.vector.tensor_scalar(out=dst, in0=dst, scalar1=t2,
                                                scalar2=c1, op0=mul, op1=add)
                        nc.vector.tensor_scalar(out=dst, in0=dst, scalar1=t2,
                                                scalar2=c0, op0=mul, op1=add)
                    # Note: sign fix — B coeffs: 0.5 - t2/24 + t2^2/720 - t2^3/40320
                    # handled by coefficient ordering above (c3*t2+c2)*t2+c1)*t2+c0
                    # negA, negB at o+15, o+16
                    nc.vector.tensor_scalar(out=prow[0:1, o + 15:o + 17],
                                            in0=prow[0:1, o + 12:o + 14],
                                            scalar1=-1.0, scalar2=None, op0=mul)
                    # skew row [0,-wz,wy, wz,0,-wx, -wy,wx,0] at cols o+21..o+29
                    nc.vector.tensor_copy(prow[0:1, o + 24:o + 25], prow[0:1, o + 2:o + 3])
                    nc.vector.tensor_copy(prow[0:1, o + 23:o + 24], prow[0:1, o + 1:o + 2])
                    nc.vector.tensor_copy(prow[0:1, o + 28:o + 29], prow[0:1, o:o + 1])
                    nc.vector.tensor_scalar(out=prow[0:1, o + 22:o + 23],
                                            in0=prow[0:1, o + 2:o + 3],
                                            scalar1=-1.0, scalar2=None, op0=mul)
                    nc.vector.tensor_scalar(out=prow[0:1, o + 26:o + 27],
                                            in0=prow[0:1, o:o + 1],
                                            scalar1=-1.0, scalar2=None, op0=mul)
                    nc.vector.tensor_scalar(out=prow[0:1, o + 27:o + 28],
                                            in0=prow[0:1, o + 1:o + 2],
                                            scalar1=-1.0, scalar2=None, op0=mul)
                    c = 16 * s
                    nc.sync.dma_start(Wm[0:3, c:c + 3], prow[0:1, o + 21:o + 30])
                    nc.sync.dma_start(Wm[0:3, c + 3:c + 4], prow[0:1, o + 3:o + 6])
                    # broadcast [t2,A,B,C,-A,-B] to partitions 0-2
                    pb_ = sps()
                    nc.tensor.matmul(pb_[0:3, 0:6], ones13[:], prow[0:1, o + 11:o + 17],
                                     start=True, stop=True)
                    nc.vector.tensor_copy(bc[0:3, 8 * s:8 * s + 6], pb_[0:3, 0:6])
                    bcs = bc[0:3, 8 * s:8 * s + 6]
                    # W2 = w w^T - t2*I
                    pw = sps()
                    nc.tensor.matmul(pw[0:3, 0:3], prow[0:1, o:o + 3],
                                     prow[0:1, o:o + 3], start=True, stop=True)
                    nc.vector.tensor_scalar(out=t2I[0:3, c:c + 3], in0=eye[:],
                                            scalar1=bc[0:3, 8 * s:8 * s + 1],
                                            scalar2=None, op0=mul)
                    nc.vector.tensor_tensor(out=W2s[0:3, c:c + 3], in0=pw[0:3, 0:3],
                                            in1=t2I[0:3, c:c + 3], op=sub)
                    # u1 = B*W2 + I ; u2 = C*W2 + I
                    nc.vector.scalar_tensor_tensor(
                        out=u1[0:3, c:c + 3], in0=W2s[0:3, c:c + 3],
                        scalar=bc[0:3, 8 * s + 2:8 * s + 3], in1=eye[:],
                        op0=mul, op1=add)
                    nc.vector.scalar_tensor_tensor(
                        out=u2[0:3, c:c + 3], in0=W2s[0:3, c:c + 3],
                        scalar=bc[0:3, 8 * s + 3:8 * s + 4], in1=eye[:],
                        op0=mul, op1=add)
                    # Rg = A*W + u1 ; RgT = -A*W + u1 ; VT = -B*W + u2
                    nc.vector.scalar_tensor_tensor(
                        out=Rg[0:3, c:c + 3], in0=Wm[0:3, c:c + 3],
                        scalar=bc[0:3, 8 * s + 1:8 * s + 2], in1=u1[0:3, c:c + 3],
                        op0=mul, op1=add)
                    nc.vector.scalar_tensor_tensor(
                        out=RgT[0:3, c:c + 3], in0=Wm[0:3, c:c + 3],
                        scalar=bc[0:3, 8 * s + 4:8 * s + 5], in1=u1[0:3, c:c + 3],
                        op0=mul, op1=add)
                    nc.vector.scalar_tensor_tensor(
                        out=VT[0:3, c:c + 3], in0=Wm[0:3, c:c + 3],
                        scalar=bc[0:3, 8 * s + 5:8 * s + 6], in1=u2[0:3, c:c + 3],
                        op0=mul, op1=add)
                    # t_new = V v + Rg t ; R_new = Rg R ; RT_new = (Rg R)^T
                    pt = sps()
                    nc.tensor.matmul(pt[0:3, 0:1], VT[0:3, c:c + 3],
                                     Wm[0:3, c + 3:c + 4], start=True, stop=False)
                    nc.tensor.matmul(pt[0:3, 0:1], RgT[0:3, c:c + 3], t[:],
                                     start=False, stop=True)
                    pR = sps()
                    nc.tensor.matmul(pR[0:3, 0:3], RgT[0:3, c:c + 3], R[:],
                                     start=True, stop=True)
                    pRT = sps()
                    nc.tensor.matmul(pRT[0:3, 0:3], R[:], RgT[0:3, c:c + 3],
                                     start=True, stop=True)
                    nc.vector.tensor_copy(t[:], pt[0:3, 0:1])
                    nc.vector.tensor_copy(R[:], pR[0:3, 0:3])
                    nc.vector.tensor_copy(RT[:], pRT[0:3, 0:3])

            for s in range(SPC):
                R, RT, t = state[s]
                nc.sync.dma_start(O[3 * s:3 * s + 3, 0:3], R[:])
                nc.sync.dma_start(O[3 * s:3 * s + 3, 3:4], t[:])
    nc.finalize()
    return nc


def _get_progs():
    if "p1" not in _BUILT:
        _BUILT["p1"] = _build_prog1()
        _BUILT["p2"] = _build_prog2()
    return _BUILT["p1"], _BUILT["p2"]


def kernel(template, source, W1, b1, W2, b2, W3, b3, W4, b4, W5, b5, dt, maxiter):
    global LAST_NS
    from concourse.bass_utils import run_bass_kernel_spmd

    template = np.asarray(template, np.float32)
    source = np.asarray(source, np.float32)
    W1 = np.asarray(W1, np.float32)
    W2 = np.asarray(W2, np.float32)
    W3 = np.asarray(W3, np.float32)
    W4 = np.asarray(W4, np.float32)
    W5 = np.asarray(W5, np.float32)
    dtv = float(np.asarray(dt).reshape(-1)[0])

    m0 = template.mean(1)  # [B,3]
    m1 = source.mean(1)

    # shared weight blocks
    W2B = np.zeros((128, 128), np.float32)
    W2B[0:64, 0:64] = W2
    W2B[64:128, 64:128] = W2
    W3B = np.zeros((128, 128), np.float32)
    W3B[0:64, 0:64] = W3
    W3B[64:128, 64:128] = W3
    W4Az = np.zeros((128, 128), np.float32)
    W4Az[0:64, :] = W4
    W4Bz = np.zeros((128, 128), np.float32)
    W4Bz[64:128, :] = W4
    W5c = np.ascontiguousarray(W5)

    # J-eval transforms (host, constant given dt)
    twists = -np.eye(6) * dtv
    G = _exp_se3_np(twists)  # [6,4,4]
    Rs = [np.eye(3)] + [G[k, :3, :3] for k in range(6)]
    vs = [np.zeros(3)] + [G[k, :3, 3] for k in range(6)]

    p1, p2 = _get_progs()

    in_maps1 = []
    for c in range(NC):
        TS = np.zeros((6, 1024), np.float32)
        L1T = np.zeros((6, 896), np.float32)
        BEFF = np.zeros((128, 7), np.float32)
        for s in range(SPC):
            b = SPC * c + s
            TS[3 * s:3 * s + 3, :] = template[b].T
            for e in range(7):
                lb = (Rs[e].T @ W1).astype(np.float32)
                L1T[3 * s:3 * s + 3, 128 * e + 64 * s:128 * e + 64 * s + 64] = lb
                te = (vs[e] - Rs[e] @ m0[b]).astype(np.float32)
                BEFF[64 * s:64 * s + 64, e] = W1.T @ te
        in_maps1.append({"TS": TS, "L1T": L1T, "BEFF": BEFF, "W2B": W2B,
                         "W3B": W3B, "W4A": W4Az, "W4B": W4Bz, "W5": W5c})

    r1 = run_bass_kernel_spmd(p1, in_maps1, list(range(NC)), trace=TRACE)
    ns1 = r1.exec_time_ns or 0

    # host: J, H, pinv
    PVs, TFs = [], []
    for c in range(NC):
        F7 = r1.results[c]["F7"].astype(np.float64)  # [128,112]
        PV = np.zeros((128, 96), np.float32)
        TFt = np.zeros((128, 16), np.float32)
        for s in range(SPC):
            fe = np.zeros((7, 1024))
            for e in range(7):
                for j in range(8):
                    fe[e, 128 * j:128 * j + 128] = F7[:, 16 * e + 8 * s + j]
            tfv = fe[0]
            J = (tfv[:, None] - fe[1:7].T) / dtv  # [1024,6]
            Hm = J.T @ J
            pinv = np.linalg.solve(Hm, J.T)  # [6,1024]
            P = (-pinv).astype(np.float32)
            for j in range(8):
                PV[:, 48 * s + 6 * j:48 * s + 6 * j + 6] = P[:, 128 * j:128 * j + 128].T
                TFt[:, 8 * s + j] = tfv[128 * j:128 * j + 128]
        PVs.append(PV)
        TFs.append(TFt)

    in_maps2 = []
    for c in range(NC):
        TS = np.zeros((6, 1024), np.float32)
        M1 = np.zeros((3, 2), np.float32)
        for s in range(SPC):
            b = SPC * c + s
            TS[3 * s:3 * s + 3, :] = source[b].T
            M1[:, s] = m1[b]
        in_maps2.append({"TS": TS, "W1": W1, "M1": M1, "PV": PVs[c],
                         "TF": TFs[c], "EYE": np.eye(3, dtype=np.float32),
                         "W2B": W2B, "W3B": W3B,
                         "W4A": W4Az, "W4B": W4Bz, "W5": W5c})

    r2 = run_bass_kernel_spmd(p2, in_maps2, list(range(NC)), trace=TRACE)
    ns2 = r2.exec_time_ns or 0
    LAST_NS = ns1 + ns2

    out = np.zeros((B, 4, 4), np.float32)
    for c in range(NC):
        O = r2.results[c]["O"]  # [6,4]
        for s in range(SPC):
            b = SPC * c + s
            R = O[3 * s:3 * s + 3, 0:3].astype(np.float64)
            t = O[3 * s:3 * s + 3, 3].astype(np.float64)
            tfin = m0[b] + t - R @ m1[b]
            out[b, :3, :3] = R.astype(np.float32)
            out[b, :3, 3] = tfin.astype(np.float32)
            out[b, 3, 3] = 1.0
    return out



# revision 18
# speedup vs baseline: 1.3696x; 1.3696x over previous
"""PointNetLK on 8 TRN2 NeuronCores — batch-parallel, 2 samples/core.

Two device programs:
  prog1: 7 PointNet feature evals (tf + 6 finite-diff Jacobian evals)
  prog2: 10 LK iterations fully on-device (feat eval, pose solve via
         precomputed -pinv, exp_se3 Taylor, SE3 state update)
Host: means, J transforms, J/H/pinv solve, final 4x4 assembly.
"""

import numpy as np

B, N, NC, SPC = 16, 1024, 8, 2
MAXITER = 10

_BUILT = {}
TRACE = False
LAST_NS = 0


def _exp_se3_np(x):
    x = np.asarray(x, np.float64)
    w, v = x[..., :3], x[..., 3:]
    t2 = (w * w).sum(-1)
    t = np.sqrt(np.maximum(t2, 1e-300))
    small = t2 < 1e-12
    A = np.where(small, 1.0 - t2 / 6.0, np.sin(t) / t)
    Bc = np.where(small, 0.5 - t2 / 24.0, (1.0 - np.cos(t)) / np.maximum(t2, 1e-300))
    C = np.where(small, 1.0 / 6.0 - t2 / 120.0, (t - np.sin(t)) / np.maximum(t2 * t, 1e-300))
    z = np.zeros_like(t2)
    wx, wy, wz = w[..., 0], w[..., 1], w[..., 2]
    W = np.stack([
        np.stack([z, -wz, wy], -1),
        np.stack([wz, z, -wx], -1),
        np.stack([-wy, wx, z], -1)], -2)
    W2 = W @ W
    I = np.eye(3)
    R = I + A[..., None, None] * W + Bc[..., None, None] * W2
    V = I + Bc[..., None, None] * W + C[..., None, None] * W2
    tv = np.einsum('...ij,...j->...i', V, v)
    out = np.zeros(x.shape[:-1] + (4, 4))
    out[..., :3, :3] = R
    out[..., :3, 3] = tv
    out[..., 3, 3] = 1.0
    return out


N_ACT_ROUTE = 8  # of the 16 L5 psum pairs, how many go ACT-copy + SBUF-reduce


def _feat_eval(nc, tc, bigps, pairps, scrp, ts, l1t_ap, beff_ap, w2, w3, w4a,
               w4b, w5, x1, x2, x3, x4a, x4b, fdst):
    import concourse.mybir as mybir
    Relu = mybir.ActivationFunctionType.Relu
    mx = mybir.AluOpType.max
    F32R = mybir.dt.float32r
    H = 512

    def mm_act(lhsT, rhs_tile, out_tile, bias):
        for h in range(2):
            p = bigps()
            nc.tensor.matmul(p[:, 0:H], lhsT,
                             rhs_tile[:, h * H:(h + 1) * H],
                             start=True, stop=True)
            nc.scalar.activation(out_tile[:, h * H:(h + 1) * H], p[:, 0:H],
                                 Relu, bias=bias)

    mm_act(l1t_ap, ts, x1, beff_ap)
    mm_act(w2[:], x1, x2, 0.0)
    mm_act(w3[:], x2, x3, 0.0)
    mm_act(w4a[:], x3, x4a, 0.0)
    mm_act(w4b[:], x3, x4b, 0.0)
    k = 0
    for s, x4 in ((0, x4a), (1, x4b)):
        for j in range(8):
            pp = pairps()
            w5j = w5[:, 128 * j:128 * (j + 1)]
            nc.tensor.matmul(pp[:, 0:H], w5j, x4[:, 0:H],
                             start=True, stop=True)
            nc.tensor.matmul(pp[:, H:2 * H], w5j, x4[:, H:2 * H],
                             start=True, stop=True)
            col = 8 * s + j
            if k % 16 < N_ACT_ROUTE:
                sc = scrp()
                nc.scalar.activation(sc[:], pp[:], Relu)
                nc.vector.tensor_reduce(fdst[:, col:col + 1], sc[:],
                                        axis=mybir.AxisListType.X, op=mx)
            else:
                nc.vector.tensor_reduce(fdst[:, col:col + 1], pp[:],
                                        axis=mybir.AxisListType.X, op=mx)
            k += 1
    # clamp at zero: max(relu(x)) == relu(max(x)); idempotent for ACT-routed cols
    nc.vector.tensor_scalar(out=fdst[:], in0=fdst[:], scalar1=0.0,
                            scalar2=None, op0=mx)


def _build_common(nc, tc, sb):
    import concourse.mybir as mybir
    F32R = mybir.dt.float32r
    ts = sb.tile([6, 1024], F32R)
    w2 = sb.tile([128, 128], F32R)
    w3 = sb.tile([128, 128], F32R)
    w4a = sb.tile([128, 128], F32R)
    w4b = sb.tile([128, 128], F32R)
    w5 = sb.tile([128, 1024], F32R)
    x1 = sb.tile([128, 1024], F32R)
    x2 = sb.tile([128, 1024], F32R)
    x3 = sb.tile([128, 1024], F32R)
    x4a = sb.tile([128, 1024], F32R)
    x4b = sb.tile([128, 1024], F32R)
    return ts, w2, w3, w4a, w4b, w5, x1, x2, x3, x4a, x4b


def _build_prog1(n_evals=7):
    import concourse.bacc as bacc
    import concourse.mybir as mybir
    import concourse.tile as tile
    F32 = mybir.dt.float32
    F32R = mybir.dt.float32r
    nc = bacc.Bacc()
    d = {}
    for name, shp in (("TS", [6, 1024]), ("L1T", [6, 896]),
                      ("W2B", [128, 128]), ("W3B", [128, 128]),
                      ("W4A", [128, 128]), ("W4B", [128, 128]),
                      ("W5", [128, 1024])):
        d[name] = nc.declare_dram_parameter(name, shp, F32R, isOutput=False)
    d["BEFF"] = nc.declare_dram_parameter("BEFF", [128, 7], F32, isOutput=False)
    F7 = nc.declare_dram_parameter("F7", [128, 112], F32, isOutput=True)

    with tile.TileContext(nc) as tc:
        with (tc.tile_pool(name="sb", bufs=1) as sb,
              tc.tile_pool(name="scr", bufs=3) as scrpool,
              tc.tile_pool(name="psb", bufs=2, space="PSUM") as psb,
              tc.tile_pool(name="psp", bufs=3, space="PSUM") as psp):
            ts, w2, w3, w4a, w4b, w5, x1, x2, x3, x4a, x4b = _build_common(nc, tc, sb)
            l1t = sb.tile([6, 896], F32R)
            beff = sb.tile([128, 7], F32)
            feats = sb.tile([128, 112], F32)
            for t_, d_ in ((ts, d["TS"]), (l1t, d["L1T"]), (beff, d["BEFF"]),
                           (w2, d["W2B"]), (w3, d["W3B"]), (w4a, d["W4A"]),
                           (w4b, d["W4B"]), (w5, d["W5"])):
                nc.sync.dma_start(t_[:], d_[:])

            def bigps():
                return psb.tile([128, 512], F32, name="bp", tag="bp")

            def pairps():
                return psp.tile([128, 1024], F32, name="pp", tag="pp")

            def scrp():
                return scrpool.tile([128, 1024], F32, name="sc", tag="sc")

            for e in range(n_evals):
                _feat_eval(nc, tc, bigps, pairps, scrp, ts,
                           l1t[:, 128 * e:128 * e + 128],
                           beff[:, e:e + 1], w2, w3, w4a, w4b, w5,
                           x1, x2, x3, x4a, x4b,
                           feats[:, 16 * e:16 * e + 16])
            nc.sync.dma_start(F7[:], feats[:])
    nc.finalize()
    return nc


def _build_prog2():
    import concourse.bacc as bacc
    import concourse.mybir as mybir
    import concourse.tile as tile
    F32 = mybir.dt.float32
    mul = mybir.AluOpType.mult
    add = mybir.AluOpType.add
    sub = mybir.AluOpType.subtract
    Copy = mybir.ActivationFunctionType.Copy
    F32R = mybir.dt.float32r
    nc = bacc.Bacc()
    d = {}
    for name, shp in (("W1", [3, 64]), ("M1", [3, 2]),
                      ("PV", [128, 96]), ("TF", [128, 16]), ("EYE", [3, 3])):
        d[name] = nc.declare_dram_parameter(name, shp, F32, isOutput=False)
    for name, shp in (("TS", [6, 1024]), ("ZL", [6, 128]),
                      ("W2B", [128, 128]), ("W3B", [128, 128]),
                      ("W4A", [128, 128]), ("W4B", [128, 128]),
                      ("W5", [128, 1024])):
        d[name] = nc.declare_dram_parameter(name, shp, F32R, isOutput=False)
    O = nc.declare_dram_parameter("O", [6, 4], F32, isOutput=True)

    with tile.TileContext(nc) as tc:
        with (tc.tile_pool(name="sb", bufs=1) as sb,
              tc.tile_pool(name="scr", bufs=3) as scrpool,
              tc.tile_pool(name="psb", bufs=2, space="PSUM") as psb,
              tc.tile_pool(name="psp", bufs=2, space="PSUM") as psp,
              tc.tile_pool(name="pss", bufs=2, space="PSUM") as pss):
            ts, w2, w3, w4a, w4b, w5, x1, x2, x3, x4a, x4b = _build_common(nc, tc, sb)
            w1 = sb.tile([3, 64], F32)
            m1 = sb.tile([3, 2], F32)
            pv = sb.tile([128, 96], F32)
            tf = sb.tile([128, 16], F32)
            feats = sb.tile([128, 16], F32)
            rr = sb.tile([128, 16], F32)
            l1t = sb.tile([6, 128], F32R)
            beff2 = sb.tile([128, 1], F32)
            stgl = sb.tile([3, 64], F32R)
            stgb = sb.tile([64, 1], F32)
            prow = sb.tile([1, 64], F32)
            ones13 = sb.tile([1, 3], F32)
            eye = sb.tile([3, 3], F32)
            bc = sb.tile([3, 16], F32)
            Wm = sb.tile([3, 32], F32)
            W2s = sb.tile([3, 32], F32)
            t2I = sb.tile([3, 32], F32)
            u1 = sb.tile([3, 32], F32)
            u2 = sb.tile([3, 32], F32)
            Rg = sb.tile([3, 32], F32)
            RgT = sb.tile([3, 32], F32)
            VT = sb.tile([3, 32], F32)
            teff = sb.tile([3, 2], F32)
            Ra = sb.tile([3, 3], F32)
            Rb = sb.tile([3, 3], F32)
            RTa = sb.tile([3, 3], F32)
            RTb = sb.tile([3, 3], F32)
            ta = sb.tile([3, 1], F32)
            tb = sb.tile([3, 1], F32)

            for t_, d_ in ((ts, d["TS"]), (w1, d["W1"]), (m1, d["M1"]),
                           (pv, d["PV"]), (tf, d["TF"]), (eye, d["EYE"]),
                           (w2, d["W2B"]), (w3, d["W3B"]), (w4a, d["W4A"]),
                           (w4b, d["W4B"]), (w5, d["W5"])):
                nc.sync.dma_start(t_[:], d_[:])

            nc.sync.dma_start(l1t[:], d["ZL"][:])
            nc.vector.memset(prow[:], 0.0)
            nc.vector.memset(ones13[:], 1.0)
            nc.vector.memset(ta[:], 0.0)
            nc.vector.memset(tb[:], 0.0)
            nc.vector.tensor_copy(Ra[:], eye[:])
            nc.vector.tensor_copy(Rb[:], eye[:])
            nc.vector.tensor_copy(RTa[:], eye[:])
            nc.vector.tensor_copy(RTb[:], eye[:])

            def sps():
                return pss.tile([64, 512], F32, name="sp", tag="sp")

            def bigps():
                return psb.tile([128, 512], F32, name="bp", tag="bp")

            def pairps():
                return psp.tile([128, 1024], F32, name="pp", tag="pp")

            def scrp():
                return scrpool.tile([128, 1024], F32, name="sc", tag="sc")

            state = [(Ra, RTa, ta), (Rb, RTb, tb)]

            for it in range(MAXITER):
                # fold est_T and mean-shift into L1 weights/bias
                for s in range(SPC):
                    R, RT, t = state[s]
                    p = sps()
                    nc.tensor.matmul(p[0:3, 0:64], R[:], w1[:], start=True, stop=True)
                    if s == 0:
                        nc.scalar.activation(l1t[0:3, 0:64], p[0:3, 0:64], Copy)
                    else:
                        nc.scalar.activation(stgl[0:3, 0:64], p[0:3, 0:64], Copy)
                        nc.sync.dma_start(l1t[3:6, 64:128], stgl[0:3, 0:64])
                    p2 = sps()
                    nc.tensor.matmul(p2[0:3, 0:1], RT[:], m1[:, s:s + 1], start=True, stop=True)
                    nc.vector.tensor_tensor(out=teff[:, s:s + 1], in0=t[:], in1=p2[0:3, 0:1], op=sub)
                    p3 = sps()
                    nc.tensor.matmul(p3[0:64, 0:1], w1[:], teff[:, s:s + 1], start=True, stop=True)
                    if s == 0:
                        nc.scalar.activation(beff2[0:64, 0:1], p3[0:64, 0:1], Copy)
                    else:
                        nc.scalar.activation(stgb[0:64, 0:1], p3[0:64, 0:1], Copy)
                        nc.sync.dma_start(beff2[64:128, 0:1], stgb[0:64, 0:1])

                _feat_eval(nc, tc, bigps, pairps, scrp, ts, l1t[:],
                           beff2[:, 0:1], w2, w3, w4a, w4b, w5,
                           x1, x2, x3, x4a, x4b, feats[:, 0:16])
                nc.vector.tensor_tensor(out=rr[:], in0=feats[:], in1=tf[:], op=sub)

                for s in range(SPC):
                    R, RT, t = state[s]
                    o = 32 * s
                    pp = sps()
                    for j in range(8):
                        nc.tensor.matmul(pp[0:1, 0:6], rr[:, 8 * s + j:8 * s + j + 1],
                                         pv[:, 48 * s + 6 * j:48 * s + 6 * j + 6],
                                         start=(j == 0), stop=(j == 7))
                    nc.vector.tensor_copy(prow[0:1, o:o + 6], pp[0:1, 0:6])
                    # t2 = |w|^2 at col o+11
                    nc.vector.tensor_tensor(out=prow[0:1, o + 8:o + 11],
                                            in0=prow[0:1, o:o + 3],
                                            in1=prow[0:1, o:o + 3], op=mul)
                    nc.vector.tensor_reduce(prow[0:1, o + 11:o + 12],
                                            prow[0:1, o + 8:o + 11],
                                            axis=mybir.AxisListType.X, op=add)
                    t2 = prow[0:1, o + 11:o + 12]
                    # Taylor A,B,C at cols o+12..o+14 (Horner in-place)
                    for col, (c3, c2, c1, c0) in (
                            (12, (-1.0 / 5040, 1.0 / 120, -1.0 / 6, 1.0)),
                            (13, (-1.0 / 40320, 1.0 / 720, -1.0 / 24, 0.5)),
                            (14, (-1.0 / 362880, 1.0 / 5040, -1.0 / 120, 1.0 / 6))):
                        dst = prow[0:1, o + col:o + col + 1]
                        nc.vector.tensor_scalar(out=dst, in0=t2, scalar1=c3,
                                                scalar2=c2, op0=mul, op1=add)
                        nc.vector.tensor_scalar(out=dst, in0=dst, scalar1=t2,
                                                scalar2=c1, op0=mul, op1=add)
                        nc.vector.tensor_scalar(out=dst, in0=dst, scalar1=t2,
                                                scalar2=c0, op0=mul, op1=add)
                    # Note: sign fix — B coeffs: 0.5 - t2/24 + t2^2/720 - t2^3/40320
                    # handled by coefficient ordering above (c3*t2+c2)*t2+c1)*t2+c0
                    # negA, negB at o+15, o+16
                    nc.vector.tensor_scalar(out=prow[0:1, o + 15:o + 17],
                                            in0=prow[0:1, o + 12:o + 14],
                                            scalar1=-1.0, scalar2=None, op0=mul)
                    # skew row [0,-wz,wy, wz,0,-wx, -wy,wx,0] at cols o+21..o+29
                    nc.vector.tensor_copy(prow[0:1, o + 24:o + 25], prow[0:1, o + 2:o + 3])
                    nc.vector.tensor_copy(prow[0:1, o + 23:o + 24], prow[0:1, o + 1:o + 2])
                    nc.vector.tensor_copy(prow[0:1, o + 28:o + 29], prow[0:1, o:o + 1])
                    nc.vector.tensor_scalar(out=prow[0:1, o + 22:o + 23],
                                            in0=prow[0:1, o + 2:o + 3],
                                            scalar1=-1.0, scalar2=None, op0=mul)
                    nc.vector.tensor_scalar(out=prow[0:1, o + 26:o + 27],
                                            in0=prow[0:1, o:o + 1],
                                            scalar1=-1.0, scalar2=None, op0=mul)
                    nc.vector.tensor_scalar(out=prow[0:1, o + 27:o + 28],
                                            in0=prow[0:1, o + 1:o + 2],
                                            scalar1=-1.0, scalar2=None, op0=mul)
                    c = 16 * s
                    nc.sync.dma_start(Wm[0:3, c:c + 3], prow[0:1, o + 21:o + 30])
                    nc.sync.dma_start(Wm[0:3, c + 3:c + 4], prow[0:1, o + 3:o + 6])
                    # broadcast [t2,A,B,C,-A,-B] to partitions 0-2
                    pb_ = sps()
                    nc.tensor.matmul(pb_[0:3, 0:6], ones13[:], prow[0:1, o + 11:o + 17],
                                     start=True, stop=True)
                    nc.vector.tensor_copy(bc[0:3, 8 * s:8 * s + 6], pb_[0:3, 0:6])
                    bcs = bc[0:3, 8 * s:8 * s + 6]
                    # W2 = w w^T - t2*I
                    pw = sps()
                    nc.tensor.matmul(pw[0:3, 0:3], prow[0:1, o:o + 3],
                                     prow[0:1, o:o + 3], start=True, stop=True)
                    nc.vector.tensor_scalar(out=t2I[0:3, c:c + 3], in0=eye[:],
                                            scalar1=bc[0:3, 8 * s:8 * s + 1],
                                            scalar2=None, op0=mul)
                    nc.vector.tensor_tensor(out=W2s[0:3, c:c + 3], in0=pw[0:3, 0:3],
                                            in1=t2I[0:3, c:c + 3], op=sub)
                    # u1 = B*W2 + I ; u2 = C*W2 + I
                    nc.vector.scalar_tensor_tensor(
                        out=u1[0:3, c:c + 3], in0=W2s[0:3, c:c + 3],
                        scalar=bc[0:3, 8 * s + 2:8 * s + 3], in1=eye[:],
                        op0=mul, op1=add)
                    nc.vector.scalar_tensor_tensor(
                        out=u2[0:3, c:c + 3], in0=W2s[0:3, c:c + 3],
                        scalar=bc[0:3, 8 * s + 3:8 * s + 4], in1=eye[:],
                        op0=mul, op1=add)
                    # Rg = A*W + u1 ; RgT = -A*W + u1 ; VT = -B*W + u2
                    nc.vector.scalar_tensor_tensor(
                        out=Rg[0:3, c:c + 3], in0=Wm[0:3, c:c + 3],
                        scalar=bc[0:3, 8 * s + 1:8 * s + 2], in1=u1[0:3, c:c + 3],
                        op0=mul, op1=add)
                    nc.vector.scalar_tensor_tensor(
                        out=RgT[0:3, c:c + 3], in0=Wm[0:3, c:c + 3],
                        scalar=bc[0:3, 8 * s + 4:8 * s + 5], in1=u1[0:3, c:c + 3],
                        op0=mul, op1=add)
                    nc.vector.scalar_tensor_tensor(
                        out=VT[0:3, c:c + 3], in0=Wm[0:3, c:c + 3],
                        scalar=bc[0:3, 8 * s + 5:8 * s + 6], in1=u2[0:3, c:c + 3],
                        op0=mul, op1=add)
                    # t_new = V v + Rg t ; R_new = Rg R ; RT_new = (Rg R)^T
                    pt = sps()
                    nc.tensor.matmul(pt[0:3, 0:1], VT[0:3, c:c + 3],
                                     Wm[0:3, c + 3:c + 4], start=True, stop=False)
                    nc.tensor.matmul(pt[0:3, 0:1], RgT[0:3, c:c + 3], t[:],
                                     start=False, stop=True)
                    pR = sps()
                    nc.tensor.matmul(pR[0:3, 0:3], RgT[0:3, c:c + 3], R[:],
                                     start=True, stop=True)
                    pRT = sps()
                    nc.tensor.matmul(pRT[0:3, 0:3], R[:], RgT[0:3, c:c + 3],
                                     start=True, stop=True)
                    nc.vector.tensor_copy(t[:], pt[0:3, 0:1])
                    nc.vector.tensor_copy(R[:], pR[0:3, 0:3])
                    nc.vector.tensor_copy(RT[:], pRT[0:3, 0:3])

            for s in range(SPC):
                R, RT, t = state[s]
                nc.sync.dma_start(O[3 * s:3 * s + 3, 0:3], R[:])
                nc.sync.dma_start(O[3 * s:3 * s + 3, 3:4], t[:])
    nc.finalize()
    return nc


def _get_progs():
    if "p1" not in _BUILT:
        _BUILT["p1"] = _build_prog1()
        _BUILT["p2"] = _build_prog2()
    return _BUILT["p1"], _BUILT["p2"]


def kernel(template, source, W1, b1, W2, b2, W3, b3, W4, b4, W5, b5, dt, maxiter):
    global LAST_NS
    from concourse.bass_utils import run_bass_kernel_spmd

    template = np.asarray(template, np.float32)
    source = np.asarray(source, np.float32)
    W1 = np.asarray(W1, np.float32)
    W2 = np.asarray(W2, np.float32)
    W3 = np.asarray(W3, np.float32)
    W4 = np.asarray(W4, np.float32)
    W5 = np.asarray(W5, np.float32)
    dtv = float(np.asarray(dt).reshape(-1)[0])

    m0 = template.mean(1)  # [B,3]
    m1 = source.mean(1)

    # shared weight blocks
    W2B = np.zeros((128, 128), np.float32)
    W2B[0:64, 0:64] = W2
    W2B[64:128, 64:128] = W2
    W3B = np.zeros((128, 128), np.float32)
    W3B[0:64, 0:64] = W3
    W3B[64:128, 64:128] = W3
    W4Az = np.zeros((128, 128), np.float32)
    W4Az[0:64, :] = W4
    W4Bz = np.zeros((128, 128), np.float32)
    W4Bz[64:128, :] = W4
    W5c = np.ascontiguousarray(W5)

    # J-eval transforms (host, constant given dt)
    twists = -np.eye(6) * dtv
    G = _exp_se3_np(twists)  # [6,4,4]
    Rs = [np.eye(3)] + [G[k, :3, :3] for k in range(6)]
    vs = [np.zeros(3)] + [G[k, :3, 3] for k in range(6)]

    p1, p2 = _get_progs()

    in_maps1 = []
    for c in range(NC):
        TS = np.zeros((6, 1024), np.float32)
        L1T = np.zeros((6, 896), np.float32)
        BEFF = np.zeros((128, 7), np.float32)
        for s in range(SPC):
            b = SPC * c + s
            TS[3 * s:3 * s + 3, :] = template[b].T
            for e in range(7):
                lb = (Rs[e].T @ W1).astype(np.float32)
                L1T[3 * s:3 * s + 3, 128 * e + 64 * s:128 * e + 64 * s + 64] = lb
                te = (vs[e] - Rs[e] @ m0[b]).astype(np.float32)
                BEFF[64 * s:64 * s + 64, e] = W1.T @ te
        in_maps1.append({"TS": TS, "L1T": L1T, "BEFF": BEFF, "W2B": W2B,
                         "W3B": W3B, "W4A": W4Az, "W4B": W4Bz, "W5": W5c})

    r1 = run_bass_kernel_spmd(p1, in_maps1, list(range(NC)), trace=TRACE)
    ns1 = r1.exec_time_ns or 0

    # host: J, H, pinv
    PVs, TFs = [], []
    for c in range(NC):
        F7 = r1.results[c]["F7"].astype(np.float64)  # [128,112]
        PV = np.zeros((128, 96), np.float32)
        TFt = np.zeros((128, 16), np.float32)
        for s in range(SPC):
            fe = np.zeros((7, 1024))
            for e in range(7):
                for j in range(8):
                    fe[e, 128 * j:128 * j + 128] = F7[:, 16 * e + 8 * s + j]
            tfv = fe[0]
            J = (tfv[:, None] - fe[1:7].T) / dtv  # [1024,6]
            Hm = J.T @ J
            pinv = np.linalg.solve(Hm, J.T)  # [6,1024]
            P = (-pinv).astype(np.float32)
            for j in range(8):
                PV[:, 48 * s + 6 * j:48 * s + 6 * j + 6] = P[:, 128 * j:128 * j + 128].T
                TFt[:, 8 * s + j] = tfv[128 * j:128 * j + 128]
        PVs.append(PV)
        TFs.append(TFt)

    in_maps2 = []
    for c in range(NC):
        TS = np.zeros((6, 1024), np.float32)
        M1 = np.zeros((3, 2), np.float32)
        for s in range(SPC):
            b = SPC * c + s
            TS[3 * s:3 * s + 3, :] = source[b].T
            M1[:, s] = m1[b]
        in_maps2.append({"TS": TS, "ZL": np.zeros((6, 128), np.float32),
                         "W1": W1, "M1": M1, "PV": PVs[c],
                         "TF": TFs[c], "EYE": np.eye(3, dtype=np.float32),
                         "W2B": W2B, "W3B": W3B,
                         "W4A": W4Az, "W4B": W4Bz, "W5": W5c})

    r2 = run_bass_kernel_spmd(p2, in_maps2, list(range(NC)), trace=TRACE)
    ns2 = r2.exec_time_ns or 0
    LAST_NS = ns1 + ns2

    out = np.zeros((B, 4, 4), np.float32)
    for c in range(NC):
        O = r2.results[c]["O"]  # [6,4]
        for s in range(SPC):
            b = SPC * c + s
            R = O[3 * s:3 * s + 3, 0:3].astype(np.float64)
            t = O[3 * s:3 * s + 3, 3].astype(np.float64)
            tfin = m0[b] + t - R @ m1[b]
            out[b, :3, :3] = R.astype(np.float32)
            out[b, :3, 3] = tfin.astype(np.float32)
            out[b, 3, 3] = 1.0
    return out



# revision 31
# speedup vs baseline: 1.3920x; 1.0164x over previous
"""PointNetLK on 8 TRN2 NeuronCores — batch-parallel, 2 samples/core.

prog1: 7 PointNet feature evals (tf + 6 finite-diff Jacobian evals), fp32r.
prog2: 10 LK iterations on-device: feat eval, pose via precomputed -pinv
       (sign-mapped into Se3-hat "seg" layout by host), SE3 exp as the
       matrix polynomial G = I + S + B*S^2 + C*S^3 on 8x8 blockdiag tiles.
Host: means, J assembly, 6x6 solve, final 4x4 assembly.

Layout: 2 samples/core stacked. Points in homogeneous form: ts8 [8,1024]
rows 0-2 = sample-a points^T, row 3 = ones, rows 4-6 = sample-b, row 7 = ones.
L1 weights in [8,128] blocks carrying rotation-folded W1 + bias row.
"""

import numpy as np

B, N, NC, SPC = 16, 1024, 8, 2
MAXITER = 10
N_TTR = 11  # of 16 L5 maxpool pairs: this many via ACT-copy + fused ttr

_BUILT = {}
TRACE = False
LAST_NS = 0


def _exp_se3_np(x):
    x = np.asarray(x, np.float64)
    w, v = x[..., :3], x[..., 3:]
    t2 = (w * w).sum(-1)
    t = np.sqrt(np.maximum(t2, 1e-300))
    small = t2 < 1e-12
    A = np.where(small, 1.0 - t2 / 6.0, np.sin(t) / t)
    Bc = np.where(small, 0.5 - t2 / 24.0, (1.0 - np.cos(t)) / np.maximum(t2, 1e-300))
    C = np.where(small, 1.0 / 6.0 - t2 / 120.0, (t - np.sin(t)) / np.maximum(t2 * t, 1e-300))
    z = np.zeros_like(t2)
    wx, wy, wz = w[..., 0], w[..., 1], w[..., 2]
    W = np.stack([
        np.stack([z, -wz, wy], -1),
        np.stack([wz, z, -wx], -1),
        np.stack([-wy, wx, z], -1)], -2)
    W2 = W @ W
    I = np.eye(3)
    R = I + A[..., None, None] * W + Bc[..., None, None] * W2
    V = I + Bc[..., None, None] * W + C[..., None, None] * W2
    tv = np.einsum('...ij,...j->...i', V, v)
    out = np.zeros(x.shape[:-1] + (4, 4))
    out[..., :3, :3] = R
    out[..., :3, 3] = tv
    out[..., 3, 3] = 1.0
    return out


def _feat_eval(nc, bigps, pairps, scrp, junkp, ts8, l18_ap,
               w2, w3, w4a, w4b, w5, x1, x2, x3, x4a, x4b, fdst):
    import concourse.mybir as mybir
    Relu = mybir.ActivationFunctionType.Relu
    mx = mybir.AluOpType.max
    H = 512

    def mm_act(lhsT, rhs_tile, out_tile):
        for h in range(2):
            p = bigps()
            nc.tensor.matmul(p[:, 0:H], lhsT, rhs_tile[:, h * H:(h + 1) * H],
                             start=True, stop=True)
            nc.scalar.activation(out_tile[:, h * H:(h + 1) * H], p[:, 0:H],
                                 Relu)

    mm_act(l18_ap, ts8, x1)
    mm_act(w2[:], x1, x2)
    mm_act(w3[:], x2, x3)
    mm_act(w4a[:], x3, x4a)
    mm_act(w4b[:], x3, x4b)
    k = 0
    for s, x4 in ((0, x4a), (1, x4b)):
        for j in range(8):
            pp = pairps()
            w5j = w5[:, 128 * j:128 * (j + 1)]
            nc.tensor.matmul(pp[:, 0:H], w5j, x4[:, 0:H],
                             start=True, stop=True)
            nc.tensor.matmul(pp[:, H:2 * H], w5j, x4[:, H:2 * H],
                             start=True, stop=True)
            col = 8 * s + j
            if k % 16 < N_TTR:
                sc = scrp()
                nc.scalar.activation(sc[:], pp[:, H:2 * H], Relu)
                jk = junkp()
                nc.vector.tensor_tensor_scan(
                    out=jk[:], data0=pp[:, 0:H], data1=sc[:],
                    initial=0.0, op0=mx, op1=mx)
                nc.vector.tensor_copy(fdst[:, col:col + 1], jk[:, H - 1:H])
            else:
                nc.vector.tensor_reduce(fdst[:, col:col + 1], pp[:],
                                        axis=mybir.AxisListType.X, op=mx)
            k += 1
    # clamp at zero (relu after max); idempotent for ttr-routed cols
    nc.vector.tensor_scalar(out=fdst[:], in0=fdst[:], scalar1=0.0,
                            scalar2=None, op0=mx)


def _build_common(nc, sb):
    import concourse.mybir as mybir
    F32R = mybir.dt.float32r
    ts8 = sb.tile([8, 1024], F32R)
    w2 = sb.tile([128, 128], F32R)
    w3 = sb.tile([128, 128], F32R)
    w4a = sb.tile([128, 128], F32R)
    w4b = sb.tile([128, 128], F32R)
    w5 = sb.tile([128, 1024], F32R)
    x1 = sb.tile([128, 1024], F32R)
    x2 = sb.tile([128, 1024], F32R)
    x3 = sb.tile([128, 1024], F32R)
    x4a = sb.tile([128, 1024], F32R)
    x4b = sb.tile([128, 1024], F32R)
    return ts8, w2, w3, w4a, w4b, w5, x1, x2, x3, x4a, x4b


def _make_pools(nc, tc):
    import concourse.mybir as mybir
    F32 = mybir.dt.float32
    ctxs = dict(
        sb=tc.tile_pool(name="sb", bufs=1),
        scr=tc.tile_pool(name="scr", bufs=3),
        junk=tc.tile_pool(name="junk", bufs=2),
        psb=tc.tile_pool(name="psb", bufs=2, space="PSUM"),
        psp=tc.tile_pool(name="psp", bufs=2, space="PSUM"),
        pss=tc.tile_pool(name="pss", bufs=2, space="PSUM"),
    )
    return ctxs


def _build_prog1(n_evals=7):
    import concourse.bacc as bacc
    import concourse.mybir as mybir
    import concourse.tile as tile
    F32 = mybir.dt.float32
    F32R = mybir.dt.float32r
    nc = bacc.Bacc()
    d = {}
    for name, shp in (("TS8", [8, 1024]), ("L1T8", [8, 896]),
                      ("W2B", [128, 128]), ("W3B", [128, 128]),
                      ("W4A", [128, 128]), ("W4B", [128, 128]),
                      ("W5", [128, 1024])):
        d[name] = nc.declare_dram_parameter(name, shp, F32R, isOutput=False)
    F7 = nc.declare_dram_parameter("F7", [128, 112], F32, isOutput=True)

    with tile.TileContext(nc) as tc:
        with (tc.tile_pool(name="sb", bufs=1) as sb,
              tc.tile_pool(name="scr", bufs=3) as scrpool,
              tc.tile_pool(name="junk", bufs=2) as junkpool,
              tc.tile_pool(name="psb", bufs=2, space="PSUM") as psb,
              tc.tile_pool(name="psp", bufs=3, space="PSUM") as psp):
            ts8, w2, w3, w4a, w4b, w5, x1, x2, x3, x4a, x4b = _build_common(nc, sb)
            l1t = sb.tile([8, 896], F32R)
            feats = sb.tile([128, 112], F32)
            for t_, d_ in ((ts8, d["TS8"]), (l1t, d["L1T8"]),
                           (w2, d["W2B"]), (w3, d["W3B"]), (w4a, d["W4A"]),
                           (w4b, d["W4B"]), (w5, d["W5"])):
                nc.sync.dma_start(t_[:], d_[:])

            def bigps():
                return psb.tile([128, 512], F32, name="bp", tag="bp")

            def pairps():
                return psp.tile([128, 1024], F32, name="pp", tag="pp")

            def scrp():
                return scrpool.tile([128, 512], F32, name="sc", tag="sc")

            def junkp():
                return junkpool.tile([128, 512], F32, name="jk", tag="jk")

            for e in range(n_evals):
                _feat_eval(nc, bigps, pairps, scrp, junkp, ts8,
                           l1t[:, 128 * e:128 * e + 128],
                           w2, w3, w4a, w4b, w5, x1, x2, x3, x4a, x4b,
                           feats[:, 16 * e:16 * e + 16])
            nc.sync.dma_start(F7[:], feats[:])
    nc.finalize()
    return nc


def _build_prog2():
    import concourse.bacc as bacc
    import concourse.mybir as mybir
    import concourse.tile as tile
    F32 = mybir.dt.float32
    F32R = mybir.dt.float32r
    mul = mybir.AluOpType.mult
    add = mybir.AluOpType.add
    Copy = mybir.ActivationFunctionType.Copy
    nc = bacc.Bacc()
    d = {}
    for name, shp in (("W1BLK8", [8, 128]), ("PVX", [128, 256]),
                      ("CSEG", [1, 32]), ("SEL2", [2, 8]), ("EYE8", [8, 8]),
                      ("ONE11", [1, 1])):
        d[name] = nc.declare_dram_parameter(name, shp, F32, isOutput=False)
    for name, shp in (("TS8", [8, 1024]),
                      ("W2B", [128, 128]), ("W3B", [128, 128]),
                      ("W4A", [128, 128]), ("W4B", [128, 128]),
                      ("W5", [128, 1024])):
        d[name] = nc.declare_dram_parameter(name, shp, F32R, isOutput=False)
    O = nc.declare_dram_parameter("O", [8, 8], F32, isOutput=True)

    with tile.TileContext(nc) as tc:
        with (tc.tile_pool(name="sb", bufs=1) as sb,
              tc.tile_pool(name="scr", bufs=3) as scrpool,
              tc.tile_pool(name="junk", bufs=2) as junkpool,
              tc.tile_pool(name="psb", bufs=2, space="PSUM") as psb,
              tc.tile_pool(name="psp", bufs=2, space="PSUM") as psp,
              tc.tile_pool(name="pss", bufs=2, space="PSUM") as pss):
            ts8, w2, w3, w4a, w4b, w5, x1, x2, x3, x4a, x4b = _build_common(nc, sb)
            w1blk = sb.tile([8, 128], F32)
            pvx = sb.tile([128, 256], F32)
            cseg = sb.tile([1, 32], F32)
            sel2 = sb.tile([2, 8], F32)
            eye8 = sb.tile([8, 8], F32)
            one11 = sb.tile([1, 1], F32)
            l18 = sb.tile([8, 128], F32R)
            feats = sb.tile([128, 16], F32)
            segSB = sb.tile([1, 32], F32)
            sq6 = sb.tile([1, 6], F32)
            t2row = sb.tile([1, 2], F32)
            t2col = sb.tile([2, 1], F32)
            bc22 = sb.tile([2, 2], F32)
            s8 = sb.tile([8, 8], F32)
            st8 = sb.tile([8, 8], F32)
            s2t = sb.tile([8, 8], F32)
            gt1 = sb.tile([8, 8], F32)
            gt2 = sb.tile([8, 8], F32)
            gts = sb.tile([8, 8], F32)
            tsb = [sb.tile([8, 8], F32, name="tsb0"),
                   sb.tile([8, 8], F32, name="tsb1")]

            for t_, d_ in ((ts8, d["TS8"]), (w1blk, d["W1BLK8"]),
                           (pvx, d["PVX"]), (cseg, d["CSEG"]),
                           (sel2, d["SEL2"]), (eye8, d["EYE8"]),
                           (one11, d["ONE11"]),
                           (w2, d["W2B"]), (w3, d["W3B"]), (w4a, d["W4A"]),
                           (w4b, d["W4B"]), (w5, d["W5"])):
                nc.sync.dma_start(t_[:], d_[:])

            nc.vector.memset(s8[:], 0.0)
            nc.vector.memset(st8[:], 0.0)
            nc.vector.tensor_copy(tsb[0][:], eye8[:])

            def bigps():
                return psb.tile([128, 512], F32, name="bp", tag="bp")

            def pairps():
                return psp.tile([128, 1024], F32, name="pp", tag="pp")

            def scrp():
                return scrpool.tile([128, 512], F32, name="sc", tag="sc")

            def junkp():
                return junkpool.tile([128, 512], F32, name="jk", tag="jk")

            def sps(shape):
                return pss.tile(shape, F32, name="sp", tag="sp")

            for it in range(MAXITER):
                Tcur = tsb[it % 2]
                Tnext = tsb[(it + 1) % 2]
                # fold est_T into L1 block: l18 = Tcur^T @ W1BLK8
                pf = sps([8, 128])
                nc.tensor.matmul(pf[:, 0:128], Tcur[:], w1blk[:],
                                 start=True, stop=True)
                nc.scalar.activation(l18[:], pf[:, 0:128], Copy)

                _feat_eval(nc, bigps, pairps, scrp, junkp, ts8, l18[:],
                           w2, w3, w4a, w4b, w5, x1, x2, x3, x4a, x4b,
                           feats[:])

                # pose in "seg" layout [1,32]: CSEG + sum_j PVX_chunk^T feats
                psg = sps([1, 32])
                for s in range(SPC):
                    sl = psg[0:1, 16 * s:16 * s + 16]
                    nc.tensor.matmul(sl, one11[:],
                                     cseg[0:1, 16 * s:16 * s + 16],
                                     start=True, stop=False,
                                     skip_group_check=True)
                    for j in range(8):
                        q = 8 * s + j
                        nc.tensor.matmul(sl, feats[:, q:q + 1],
                                         pvx[:, 16 * q:16 * q + 16],
                                         start=False, stop=(j == 7),
                                         skip_group_check=True)
                nc.vector.tensor_copy(segSB[:], psg[0:1, 0:32])

                # S-hat packs via strided SBUF->SBUF DMAs
                for s_ in range(2):
                    o = 16 * s_
                    nc.sync.dma_start(
                        s8[4 * s_:4 * s_ + 3, 4 * s_:4 * s_ + 4],
                        segSB[0:1, o:o + 12])
                pst = sps([8, 8])
                nc.tensor.transpose(pst[0:8, 0:8], s8[:], eye8[:])
                nc.vector.tensor_copy(st8[:], pst[0:8, 0:8])

                # t2 = |w|^2 per sample from seg extras (slots 12-14, 28-30)
                nc.vector.tensor_tensor(
                    out=sq6[:].rearrange("p (a c) -> p a c", a=2),
                    in0=segSB[:].rearrange("p (a c) -> p a c", a=2, c=16)[:, :, 12:15],
                    in1=segSB[:].rearrange("p (a c) -> p a c", a=2, c=16)[:, :, 12:15],
                    op=mul)
                nc.vector.tensor_reduce(
                    t2row[:], sq6[:].rearrange("p (a c) -> p a c", a=2),
                    axis=mybir.AxisListType.X, op=add)
                pt2 = sps([2, 1])
                nc.tensor.matmul(pt2[0:2, 0:1], t2row[:], one11[:],
                                 start=True, stop=True)
                nc.vector.tensor_copy(t2col[:], pt2[0:2, 0:1])
                # Horner for B (col 0) and C (col 1) on [2,1]
                for col, (c3, c2, c1, c0) in (
                        (0, (-1.0 / 40320, 1.0 / 720, -1.0 / 24, 0.5)),
                        (1, (-1.0 / 362880, 1.0 / 5040, -1.0 / 120, 1.0 / 6))):
                    dst = bc22[0:2, col:col + 1]
                    nc.vector.tensor_scalar(out=dst, in0=t2col[:],
                                            scalar1=c3, scalar2=c2,
                                            op0=mul, op1=add)
                    nc.vector.tensor_scalar(out=dst, in0=dst,
                                            scalar1=t2col[:], scalar2=c1,
                                            op0=mul, op1=add)
                    nc.vector.tensor_scalar(out=dst, in0=dst,
                                            scalar1=t2col[:], scalar2=c0,
                                            op0=mul, op1=add)
                pbc = sps([8, 2])
                nc.tensor.matmul(pbc[0:8, 0:2], sel2[:], bc22[:],
                                 start=True, stop=True)

                # (S^2)^T and (S^3)^T
                ps2 = sps([8, 8])
                nc.tensor.matmul(ps2[0:8, 0:8], s8[:], st8[:],
                                 start=True, stop=True)
                nc.vector.tensor_copy(s2t[:], ps2[0:8, 0:8])
                ps3 = sps([8, 8])
                nc.tensor.matmul(ps3[0:8, 0:8], s8[:], s2t[:],
                                 start=True, stop=True)
                # G^T = I + S^T + B (S^2)^T + C (S^3)^T
                nc.vector.scalar_tensor_tensor(
                    out=gt1[:], in0=ps2[0:8, 0:8], scalar=pbc[0:8, 0:1],
                    in1=st8[:], op0=mul, op1=add)
                nc.vector.scalar_tensor_tensor(
                    out=gt2[:], in0=ps3[0:8, 0:8], scalar=pbc[0:8, 1:2],
                    in1=eye8[:], op0=mul, op1=add)
                nc.vector.tensor_tensor(out=gts[:], in0=gt1[:], in1=gt2[:],
                                        op=add)
                # T_next = G @ T_cur
                pT = sps([8, 8])
                nc.tensor.matmul(pT[0:8, 0:8], gts[:], Tcur[:],
                                 start=True, stop=True)
                nc.vector.tensor_copy(Tnext[:], pT[0:8, 0:8])

            nc.sync.dma_start(O[:], tsb[MAXITER % 2][:])
    nc.finalize()
    return nc


def _get_progs():
    if "p1" not in _BUILT:
        _BUILT["p1"] = _build_prog1()
        _BUILT["p2"] = _build_prog2()
    return _BUILT["p1"], _BUILT["p2"]


# seg slot -> (pose component k, sign); slots 0,5,10,15 are zero
_SEG_MAP = {1: (2, -1.0), 2: (1, 1.0), 3: (3, 1.0),
            4: (2, 1.0), 6: (0, -1.0), 7: (4, 1.0),
            8: (1, -1.0), 9: (0, 1.0), 11: (5, 1.0),
            12: (0, 1.0), 13: (1, 1.0), 14: (2, 1.0)}


def kernel(template, source, W1, b1, W2, b2, W3, b3, W4, b4, W5, b5, dt, maxiter):
    global LAST_NS
    from concourse.bass_utils import run_bass_kernel_spmd

    template = np.asarray(template, np.float32)
    source = np.asarray(source, np.float32)
    W1 = np.asarray(W1, np.float64)
    W2 = np.asarray(W2, np.float32)
    W3 = np.asarray(W3, np.float32)
    W4 = np.asarray(W4, np.float32)
    W5 = np.asarray(W5, np.float32)
    dtv = float(np.asarray(dt).reshape(-1)[0])

    m0 = template.mean(1)  # [B,3]
    m1 = source.mean(1)

    # shared weight blocks
    W2B = np.zeros((128, 128), np.float32)
    W2B[0:64, 0:64] = W2
    W2B[64:128, 64:128] = W2
    W3B = np.zeros((128, 128), np.float32)
    W3B[0:64, 0:64] = W3
    W3B[64:128, 64:128] = W3
    W4Az = np.zeros((128, 128), np.float32)
    W4Az[0:64, :] = W4
    W4Bz = np.zeros((128, 128), np.float32)
    W4Bz[64:128, :] = W4
    W5c = np.ascontiguousarray(W5)

    # J-eval transforms (host, constant given dt)
    twists = -np.eye(6) * dtv
    G = _exp_se3_np(twists)  # [6,4,4]
    Rs = [np.eye(3)] + [G[k, :3, :3] for k in range(6)]
    vs = [np.zeros(3)] + [G[k, :3, 3] for k in range(6)]

    p1, p2 = _get_progs()

    in_maps1 = []
    for c in range(NC):
        TS8 = np.zeros((8, 1024), np.float32)
        L1T8 = np.zeros((8, 896), np.float32)
        for s in range(SPC):
            b = SPC * c + s
            TS8[4 * s:4 * s + 3, :] = (template[b] - m0[b]).T
            TS8[4 * s + 3, :] = 1.0
            for e in range(7):
                lb = (Rs[e].T @ W1).astype(np.float32)
                L1T8[4 * s:4 * s + 3, 128 * e + 64 * s:128 * e + 64 * s + 64] = lb
                L1T8[4 * s + 3, 128 * e + 64 * s:128 * e + 64 * s + 64] = \
                    (W1.T @ vs[e]).astype(np.float32)
        in_maps1.append({"TS8": TS8, "L1T8": L1T8, "W2B": W2B,
                         "W3B": W3B, "W4A": W4Az, "W4B": W4Bz, "W5": W5c})

    r1 = run_bass_kernel_spmd(p1, in_maps1, list(range(NC)), trace=TRACE)
    ns1 = r1.exec_time_ns or 0

    # host: J, H, pinv, and seg-mapped PVX/CSEG
    PVXs, CSEGs = [], []
    for c in range(NC):
        F7 = r1.results[c]["F7"].astype(np.float64)  # [128,112]
        PVX = np.zeros((128, 256), np.float32)
        CSEG = np.zeros((1, 32), np.float32)
        for s in range(SPC):
            fe = np.zeros((7, 1024))
            for e in range(7):
                for j in range(8):
                    fe[e, 128 * j:128 * j + 128] = F7[:, 16 * e + 8 * s + j]
            tfv = fe[0]
            J = (tfv[:, None] - fe[1:7].T) / dtv  # [1024,6]
            Hm = J.T @ J
            pinv = np.linalg.solve(Hm, J.T)  # [6,1024]
            P = -pinv          # pose = P @ sf + cvec
            cvec = pinv @ tfv  # [6]
            for j in range(8):
                q = 8 * s + j
                Pj = P[:, 128 * j:128 * j + 128]  # [6,128]
                for slot, (k, sgn) in _SEG_MAP.items():
                    PVX[:, 16 * q + slot] = sgn * Pj[k]
            for slot, (k, sgn) in _SEG_MAP.items():
                CSEG[0, 16 * s + slot] = sgn * cvec[k]
        PVXs.append(PVX)
        CSEGs.append(CSEG)

    W1BLK8 = np.zeros((8, 128), np.float32)
    W1BLK8[0:3, 0:64] = W1.astype(np.float32)
    W1BLK8[4:7, 64:128] = W1.astype(np.float32)
    SEL2 = np.zeros((2, 8), np.float32)
    SEL2[0, 0:4] = 1.0
    SEL2[1, 4:8] = 1.0

    in_maps2 = []
    for c in range(NC):
        TS8 = np.zeros((8, 1024), np.float32)
        for s in range(SPC):
            b = SPC * c + s
            TS8[4 * s:4 * s + 3, :] = (source[b] - m1[b]).T
            TS8[4 * s + 3, :] = 1.0
        in_maps2.append({"TS8": TS8, "W1BLK8": W1BLK8, "PVX": PVXs[c],
                         "CSEG": CSEGs[c], "SEL2": SEL2,
                         "EYE8": np.eye(8, dtype=np.float32),
                         "ONE11": np.ones((1, 1), np.float32),
                         "W2B": W2B, "W3B": W3B,
                         "W4A": W4Az, "W4B": W4Bz, "W5": W5c})

    r2 = run_bass_kernel_spmd(p2, in_maps2, list(range(NC)), trace=TRACE)
    ns2 = r2.exec_time_ns or 0
    LAST_NS = ns1 + ns2

    out = np.zeros((B, 4, 4), np.float32)
    for c in range(NC):
        O = r2.results[c]["O"]  # [8,8]
        for s in range(SPC):
            b = SPC * c + s
            R = O[4 * s:4 * s + 3, 4 * s:4 * s + 3].astype(np.float64)
            t = O[4 * s:4 * s + 3, 4 * s + 3].astype(np.float64)
            tfin = m0[b] + t - R @ m1[b]
            out[b, :3, :3] = R.astype(np.float32)
            out[b, :3, 3] = tfin.astype(np.float32)
            out[b, 3, 3] = 1.0
    return out


# revision 42
# speedup vs baseline: 1.6069x; 1.1544x over previous
"""PointNetLK on 8 TRN2 NeuronCores — batch-parallel, 2 samples/core.

prog1: 7 PointNet feature evals (tf + 6 finite-diff Jacobian evals), fp32r.
prog2: 10 LK iterations on-device: feat eval, pose via precomputed -pinv
       (sign-mapped into Se3-hat "seg" layout by host), SE3 exp as the
       matrix polynomial G = I + S + B*S^2 + C*S^3 on 8x8 blockdiag tiles.
Host: means, J assembly, 6x6 solve, final 4x4 assembly.

Layout: 2 samples/core stacked. Points in homogeneous form: ts8 [8,1024]
rows 0-2 = sample-a points^T, row 3 = ones, rows 4-6 = sample-b, row 7 = ones.
L1 weights in [8,128] blocks carrying rotation-folded W1 + bias row.
"""

import numpy as np

B, N, NC, SPC = 16, 1024, 8, 2
MAXITER = 10
N_TTR = 11  # of 16 L5 maxpool pairs: this many via ACT-copy + fused ttr

_BUILT = {}
TRACE = False
LAST_NS = 0


def _exp_se3_np(x):
    x = np.asarray(x, np.float64)
    w, v = x[..., :3], x[..., 3:]
    t2 = (w * w).sum(-1)
    t = np.sqrt(np.maximum(t2, 1e-300))
    small = t2 < 1e-12
    A = np.where(small, 1.0 - t2 / 6.0, np.sin(t) / t)
    Bc = np.where(small, 0.5 - t2 / 24.0, (1.0 - np.cos(t)) / np.maximum(t2, 1e-300))
    C = np.where(small, 1.0 / 6.0 - t2 / 120.0, (t - np.sin(t)) / np.maximum(t2 * t, 1e-300))
    z = np.zeros_like(t2)
    wx, wy, wz = w[..., 0], w[..., 1], w[..., 2]
    W = np.stack([
        np.stack([z, -wz, wy], -1),
        np.stack([wz, z, -wx], -1),
        np.stack([-wy, wx, z], -1)], -2)
    W2 = W @ W
    I = np.eye(3)
    R = I + A[..., None, None] * W + Bc[..., None, None] * W2
    V = I + Bc[..., None, None] * W + C[..., None, None] * W2
    tv = np.einsum('...ij,...j->...i', V, v)
    out = np.zeros(x.shape[:-1] + (4, 4))
    out[..., :3, :3] = R
    out[..., :3, 3] = tv
    out[..., 3, 3] = 1.0
    return out


def _feat_eval(nc, bigps, pairps, ts8, l18_ap,
               w2, w3, w4a, w4b, w5, x1, x2, x3, x4a, x4b, fdst):
    import concourse.mybir as mybir
    Relu = mybir.ActivationFunctionType.Relu
    mx = mybir.AluOpType.max
    H = 512

    def mm_act(lhsT, rhs_tile, out_tile):
        for h in range(2):
            p = bigps()
            nc.tensor.matmul(p[:, 0:H], lhsT, rhs_tile[:, h * H:(h + 1) * H],
                             start=True, stop=True)
            nc.scalar.activation(out_tile[:, h * H:(h + 1) * H], p[:, 0:H],
                                 Relu)

    mm_act(l18_ap, ts8, x1)
    mm_act(w2[:], x1, x2)
    mm_act(w3[:], x2, x3)
    mm_act(w4a[:], x3, x4a)
    mm_act(w4b[:], x3, x4b)
    for s, x4 in ((0, x4a), (1, x4b)):
        for j in range(8):
            pp = pairps()
            w5j = w5[:, 128 * j:128 * (j + 1)]
            nc.tensor.matmul(pp[:, 0:H], w5j, x4[:, 0:H],
                             start=True, stop=True)
            nc.tensor.matmul(pp[:, H:2 * H], w5j, x4[:, H:2 * H],
                             start=True, stop=True)
            col = 8 * s + j
            nc.vector.tensor_reduce(fdst[:, col:col + 1], pp[:],
                                    axis=mybir.AxisListType.X, op=mx)
    # clamp at zero (relu after max over all points)
    nc.vector.tensor_scalar(out=fdst[:], in0=fdst[:], scalar1=0.0,
                            scalar2=None, op0=mx)


def _build_common(nc, sb, dt_):
    ts8 = sb.tile([8, 1024], dt_)
    w2 = sb.tile([128, 128], dt_)
    w3 = sb.tile([128, 128], dt_)
    w4a = sb.tile([128, 128], dt_)
    w4b = sb.tile([128, 128], dt_)
    w5 = sb.tile([128, 1024], dt_)
    x1 = sb.tile([128, 1024], dt_)
    x2 = sb.tile([128, 1024], dt_)
    x3 = sb.tile([128, 1024], dt_)
    x4a = sb.tile([128, 1024], dt_)
    x4b = sb.tile([128, 1024], dt_)
    return ts8, w2, w3, w4a, w4b, w5, x1, x2, x3, x4a, x4b


def _make_pools(nc, tc):
    import concourse.mybir as mybir
    F32 = mybir.dt.float32
    ctxs = dict(
        sb=tc.tile_pool(name="sb", bufs=1),
        scr=tc.tile_pool(name="scr", bufs=3),
        junk=tc.tile_pool(name="junk", bufs=2),
        psb=tc.tile_pool(name="psb", bufs=2, space="PSUM"),
        psp=tc.tile_pool(name="psp", bufs=2, space="PSUM"),
        pss=tc.tile_pool(name="pss", bufs=2, space="PSUM"),
    )
    return ctxs


def _build_prog1(n_evals=7):
    import concourse.bacc as bacc
    import concourse.mybir as mybir
    import concourse.tile as tile
    F32 = mybir.dt.float32
    F32R = mybir.dt.float32r
    nc = bacc.Bacc()
    d = {}
    for name, shp in (("TS8", [8, 1024]), ("L1T8", [8, 896]),
                      ("W2B", [128, 128]), ("W3B", [128, 128]),
                      ("W4A", [128, 128]), ("W4B", [128, 128]),
                      ("W5", [128, 1024])):
        d[name] = nc.declare_dram_parameter(name, shp, F32R, isOutput=False)
    F7 = nc.declare_dram_parameter("F7", [128, 112], F32, isOutput=True)

    with tile.TileContext(nc) as tc:
        with (tc.tile_pool(name="sb", bufs=1) as sb,
              tc.tile_pool(name="psb", bufs=2, space="PSUM") as psb,
              tc.tile_pool(name="psp", bufs=3, space="PSUM") as psp):
            ts8, w2, w3, w4a, w4b, w5, x1, x2, x3, x4a, x4b = _build_common(nc, sb, F32R)
            l1t = sb.tile([8, 896], F32R)
            feats = sb.tile([128, 112], F32)
            for t_, d_ in ((ts8, d["TS8"]), (l1t, d["L1T8"]),
                           (w2, d["W2B"]), (w3, d["W3B"]), (w4a, d["W4A"]),
                           (w4b, d["W4B"]), (w5, d["W5"])):
                nc.sync.dma_start(t_[:], d_[:])

            def bigps():
                return psb.tile([128, 512], F32, name="bp", tag="bp")

            def pairps():
                return psp.tile([128, 1024], F32, name="pp", tag="pp")

            for e in range(n_evals):
                _feat_eval(nc, bigps, pairps, ts8,
                           l1t[:, 128 * e:128 * e + 128],
                           w2, w3, w4a, w4b, w5, x1, x2, x3, x4a, x4b,
                           feats[:, 16 * e:16 * e + 16])
            nc.sync.dma_start(F7[:], feats[:])
    nc.finalize()
    return nc


def _build_prog2():
    import concourse.bacc as bacc
    import concourse.mybir as mybir
    import concourse.tile as tile
    F32 = mybir.dt.float32
    F32R = mybir.dt.float32r
    mul = mybir.AluOpType.mult
    add = mybir.AluOpType.add
    Copy = mybir.ActivationFunctionType.Copy
    nc = bacc.Bacc()
    d = {}
    for name, shp in (("W1BLK8", [8, 128]), ("PVX", [128, 256]),
                      ("CSEG", [1, 32]), ("SEL2", [2, 8]), ("EYE8", [8, 8]),
                      ("ONE11", [1, 1])):
        d[name] = nc.declare_dram_parameter(name, shp, F32, isOutput=False)
    BF16 = mybir.dt.bfloat16
    for name, shp in (("TS8", [8, 1024]),
                      ("W2B", [128, 128]), ("W3B", [128, 128]),
                      ("W4A", [128, 128]), ("W4B", [128, 128]),
                      ("W5", [128, 1024])):
        d[name] = nc.declare_dram_parameter(name, shp, BF16, isOutput=False)
    O = nc.declare_dram_parameter("O", [8, 8], F32, isOutput=True)

    with tile.TileContext(nc) as tc:
        with (tc.tile_pool(name="sb", bufs=1) as sb,
              tc.tile_pool(name="psb", bufs=2, space="PSUM") as psb,
              tc.tile_pool(name="psp", bufs=2, space="PSUM") as psp,
              tc.tile_pool(name="pss", bufs=2, space="PSUM") as pss):
            ts8, w2, w3, w4a, w4b, w5, x1, x2, x3, x4a, x4b = _build_common(nc, sb, BF16)
            w1blk = sb.tile([8, 128], F32)
            pvx = sb.tile([128, 256], F32)
            cseg = sb.tile([1, 32], F32)
            sel2 = sb.tile([2, 8], F32)
            eye8 = sb.tile([8, 8], F32)
            one11 = sb.tile([1, 1], F32)
            l18 = sb.tile([8, 128], BF16)
            feats = sb.tile([128, 16], F32)
            segSB = sb.tile([1, 32], F32)
            sq6 = sb.tile([1, 6], F32)
            t2row = sb.tile([1, 2], F32)
            t2col = sb.tile([2, 1], F32)
            bc22 = sb.tile([2, 2], F32)
            s8 = sb.tile([8, 8], F32)
            st8 = sb.tile([8, 8], F32)
            s2t = sb.tile([8, 8], F32)
            gt1 = sb.tile([8, 8], F32)
            gt2 = sb.tile([8, 8], F32)
            gts = sb.tile([8, 8], F32)
            tsb = [sb.tile([8, 8], F32, name="tsb0"),
                   sb.tile([8, 8], F32, name="tsb1")]

            for t_, d_ in ((ts8, d["TS8"]), (w1blk, d["W1BLK8"]),
                           (pvx, d["PVX"]), (cseg, d["CSEG"]),
                           (sel2, d["SEL2"]), (eye8, d["EYE8"]),
                           (one11, d["ONE11"]),
                           (w2, d["W2B"]), (w3, d["W3B"]), (w4a, d["W4A"]),
                           (w4b, d["W4B"]), (w5, d["W5"])):
                nc.sync.dma_start(t_[:], d_[:])

            nc.vector.memset(s8[:], 0.0)
            nc.vector.memset(st8[:], 0.0)
            nc.vector.tensor_copy(tsb[0][:], eye8[:])

            def bigps():
                return psb.tile([128, 512], F32, name="bp", tag="bp")

            def pairps():
                return psp.tile([128, 1024], F32, name="pp", tag="pp")

            def sps(shape):
                return pss.tile(shape, F32, name="sp", tag="sp")

            for it in range(MAXITER):
                Tcur = tsb[it % 2]
                Tnext = tsb[(it + 1) % 2]
                # fold est_T into L1 block: l18 = Tcur^T @ W1BLK8
                pf = sps([8, 128])
                nc.tensor.matmul(pf[:, 0:128], Tcur[:], w1blk[:],
                                 start=True, stop=True)
                nc.scalar.activation(l18[:], pf[:, 0:128], Copy)

                _feat_eval(nc, bigps, pairps, ts8, l18[:],
                           w2, w3, w4a, w4b, w5, x1, x2, x3, x4a, x4b,
                           feats[:])

                # pose in "seg" layout [1,32]: CSEG + sum_j PVX_chunk^T feats
                psg = sps([1, 32])
                for s in range(SPC):
                    sl = psg[0:1, 16 * s:16 * s + 16]
                    nc.tensor.matmul(sl, one11[:],
                                     cseg[0:1, 16 * s:16 * s + 16],
                                     start=True, stop=False,
                                     skip_group_check=True)
                    for j in range(8):
                        q = 8 * s + j
                        nc.tensor.matmul(sl, feats[:, q:q + 1],
                                         pvx[:, 16 * q:16 * q + 16],
                                         start=False, stop=(j == 7),
                                         skip_group_check=True)
                # S-hat blocks: copy pose row to SBUF, strided DMAs into s8
                nc.vector.tensor_copy(segSB[:], psg[0:1, 0:32])
                for s_ in range(2):
                    o = 16 * s_
                    nc.sync.dma_start(
                        s8[4 * s_:4 * s_ + 3, 4 * s_:4 * s_ + 4],
                        segSB[0:1, o:o + 12])
                pst = sps([8, 8])
                nc.tensor.transpose(pst[0:8, 0:8], s8[:], eye8[:])
                nc.vector.tensor_copy(st8[:], pst[0:8, 0:8])

                # t2 = |w|^2 per sample from seg extras (slots 12-14, 28-30)
                nc.scalar.square(
                    sq6[:].rearrange("p (a c) -> p a c", a=2),
                    psg[0:1, 0:32].rearrange("p (a c) -> p a c", a=2, c=16)[:, :, 12:15])
                nc.vector.tensor_reduce(
                    t2row[:], sq6[:].rearrange("p (a c) -> p a c", a=2),
                    axis=mybir.AxisListType.X, op=add)
                pt2 = sps([2, 1])
                nc.tensor.matmul(pt2[0:2, 0:1], t2row[:], one11[:],
                                 start=True, stop=True)
                nc.vector.tensor_copy(t2col[:], pt2[0:2, 0:1])
                # Horner for B (col 0) and C (col 1) on [2,1]
                for col, (c3, c2, c1, c0) in (
                        (0, (-1.0 / 40320, 1.0 / 720, -1.0 / 24, 0.5)),
                        (1, (-1.0 / 362880, 1.0 / 5040, -1.0 / 120, 1.0 / 6))):
                    dst = bc22[0:2, col:col + 1]
                    nc.vector.tensor_scalar(out=dst, in0=t2col[:],
                                            scalar1=c3, scalar2=c2,
                                            op0=mul, op1=add)
                    nc.vector.tensor_scalar(out=dst, in0=dst,
                                            scalar1=t2col[:], scalar2=c1,
                                            op0=mul, op1=add)
                    nc.vector.tensor_scalar(out=dst, in0=dst,
                                            scalar1=t2col[:], scalar2=c0,
                                            op0=mul, op1=add)
                pbc = sps([8, 2])
                nc.tensor.matmul(pbc[0:8, 0:2], sel2[:], bc22[:],
                                 start=True, stop=True)

                # (S^2)^T and (S^3)^T
                ps2 = sps([8, 8])
                nc.tensor.matmul(ps2[0:8, 0:8], s8[:], st8[:],
                                 start=True, stop=True)
                nc.vector.tensor_copy(s2t[:], ps2[0:8, 0:8])
                ps3 = sps([8, 8])
                nc.tensor.matmul(ps3[0:8, 0:8], s8[:], s2t[:],
                                 start=True, stop=True)
                # G^T = I + S^T + B (S^2)^T + C (S^3)^T
                nc.vector.scalar_tensor_tensor(
                    out=gt1[:], in0=ps2[0:8, 0:8], scalar=pbc[0:8, 0:1],
                    in1=st8[:], op0=mul, op1=add)
                nc.vector.scalar_tensor_tensor(
                    out=gt2[:], in0=ps3[0:8, 0:8], scalar=pbc[0:8, 1:2],
                    in1=eye8[:], op0=mul, op1=add)
                nc.vector.tensor_tensor(out=gts[:], in0=gt1[:], in1=gt2[:],
                                        op=add)
                # T_next = G @ T_cur
                pT = sps([8, 8])
                nc.tensor.matmul(pT[0:8, 0:8], gts[:], Tcur[:],
                                 start=True, stop=True)
                nc.vector.tensor_copy(Tnext[:], pT[0:8, 0:8])

            nc.sync.dma_start(O[:], tsb[MAXITER % 2][:])
    nc.finalize()
    return nc


def _get_progs():
    if "p1" not in _BUILT:
        _BUILT["p1"] = _build_prog1()
        _BUILT["p2"] = _build_prog2()
    return _BUILT["p1"], _BUILT["p2"]


# seg slot -> (pose component k, sign); slots 0,5,10,15 are zero
_SEG_MAP = {1: (2, -1.0), 2: (1, 1.0), 3: (3, 1.0),
            4: (2, 1.0), 6: (0, -1.0), 7: (4, 1.0),
            8: (1, -1.0), 9: (0, 1.0), 11: (5, 1.0),
            12: (0, 1.0), 13: (1, 1.0), 14: (2, 1.0)}


def kernel(template, source, W1, b1, W2, b2, W3, b3, W4, b4, W5, b5, dt, maxiter):
    global LAST_NS
    from concourse.bass_utils import run_bass_kernel_spmd

    template = np.asarray(template, np.float32)
    source = np.asarray(source, np.float32)
    W1 = np.asarray(W1, np.float64)
    W2 = np.asarray(W2, np.float32)
    W3 = np.asarray(W3, np.float32)
    W4 = np.asarray(W4, np.float32)
    W5 = np.asarray(W5, np.float32)
    dtv = float(np.asarray(dt).reshape(-1)[0])

    m0 = template.mean(1)  # [B,3]
    m1 = source.mean(1)

    # shared weight blocks
    W2B = np.zeros((128, 128), np.float32)
    W2B[0:64, 0:64] = W2
    W2B[64:128, 64:128] = W2
    W3B = np.zeros((128, 128), np.float32)
    W3B[0:64, 0:64] = W3
    W3B[64:128, 64:128] = W3
    W4Az = np.zeros((128, 128), np.float32)
    W4Az[0:64, :] = W4
    W4Bz = np.zeros((128, 128), np.float32)
    W4Bz[64:128, :] = W4
    W5c = np.ascontiguousarray(W5)

    # J-eval transforms (host, constant given dt)
    twists = -np.eye(6) * dtv
    G = _exp_se3_np(twists)  # [6,4,4]
    Rs = [np.eye(3)] + [G[k, :3, :3] for k in range(6)]
    vs = [np.zeros(3)] + [G[k, :3, 3] for k in range(6)]

    p1, p2 = _get_progs()

    in_maps1 = []
    for c in range(NC):
        TS8 = np.zeros((8, 1024), np.float32)
        L1T8 = np.zeros((8, 896), np.float32)
        for s in range(SPC):
            b = SPC * c + s
            TS8[4 * s:4 * s + 3, :] = (template[b] - m0[b]).T
            TS8[4 * s + 3, :] = 1.0
            for e in range(7):
                lb = (Rs[e].T @ W1).astype(np.float32)
                L1T8[4 * s:4 * s + 3, 128 * e + 64 * s:128 * e + 64 * s + 64] = lb
                L1T8[4 * s + 3, 128 * e + 64 * s:128 * e + 64 * s + 64] = \
                    (W1.T @ vs[e]).astype(np.float32)
        in_maps1.append({"TS8": TS8, "L1T8": L1T8, "W2B": W2B,
                         "W3B": W3B, "W4A": W4Az, "W4B": W4Bz, "W5": W5c})

    r1 = run_bass_kernel_spmd(p1, in_maps1, list(range(NC)), trace=TRACE)
    ns1 = r1.exec_time_ns or 0

    # host: J, H, pinv, and seg-mapped PVX/CSEG
    PVXs, CSEGs = [], []
    for c in range(NC):
        F7 = r1.results[c]["F7"].astype(np.float64)  # [128,112]
        PVX = np.zeros((128, 256), np.float32)
        CSEG = np.zeros((1, 32), np.float32)
        for s in range(SPC):
            fe = np.zeros((7, 1024))
            for e in range(7):
                for j in range(8):
                    fe[e, 128 * j:128 * j + 128] = F7[:, 16 * e + 8 * s + j]
            tfv = fe[0]
            J = (tfv[:, None] - fe[1:7].T) / dtv  # [1024,6]
            Hm = J.T @ J
            pinv = np.linalg.solve(Hm, J.T)  # [6,1024]
            P = -pinv          # pose = P @ sf + cvec
            cvec = pinv @ tfv  # [6]
            for j in range(8):
                q = 8 * s + j
                Pj = P[:, 128 * j:128 * j + 128]  # [6,128]
                for slot, (k, sgn) in _SEG_MAP.items():
                    PVX[:, 16 * q + slot] = sgn * Pj[k]
            for slot, (k, sgn) in _SEG_MAP.items():
                CSEG[0, 16 * s + slot] = sgn * cvec[k]
        PVXs.append(PVX)
        CSEGs.append(CSEG)

    W1BLK8 = np.zeros((8, 128), np.float32)
    W1BLK8[0:3, 0:64] = W1.astype(np.float32)
    W1BLK8[4:7, 64:128] = W1.astype(np.float32)
    SEL2 = np.zeros((2, 8), np.float32)
    SEL2[0, 0:4] = 1.0
    SEL2[1, 4:8] = 1.0

    import ml_dtypes
    bf = ml_dtypes.bfloat16
    in_maps2 = []
    for c in range(NC):
        TS8 = np.zeros((8, 1024), np.float32)
        for s in range(SPC):
            b = SPC * c + s
            TS8[4 * s:4 * s + 3, :] = (source[b] - m1[b]).T
            TS8[4 * s + 3, :] = 1.0
        in_maps2.append({"TS8": TS8.astype(bf), "W1BLK8": W1BLK8,
                         "PVX": PVXs[c],
                         "CSEG": CSEGs[c], "SEL2": SEL2,
                         "EYE8": np.eye(8, dtype=np.float32),
                         "ONE11": np.ones((1, 1), np.float32),
                         "W2B": W2B.astype(bf), "W3B": W3B.astype(bf),
                         "W4A": W4Az.astype(bf), "W4B": W4Bz.astype(bf),
                         "W5": W5c.astype(bf)})

    r2 = run_bass_kernel_spmd(p2, in_maps2, list(range(NC)), trace=TRACE)
    ns2 = r2.exec_time_ns or 0
    LAST_NS = ns1 + ns2

    out = np.zeros((B, 4, 4), np.float32)
    for c in range(NC):
        O = r2.results[c]["O"]  # [8,8]
        for s in range(SPC):
            b = SPC * c + s
            R = O[4 * s:4 * s + 3, 4 * s:4 * s + 3].astype(np.float64)
            t = O[4 * s:4 * s + 3, 4 * s + 3].astype(np.float64)
            tfin = m0[b] + t - R @ m1[b]
            out[b, :3, :3] = R.astype(np.float32)
            out[b, :3, 3] = tfin.astype(np.float32)
            out[b, 3, 3] = 1.0
    return out


# revision 48
# speedup vs baseline: 1.6599x; 1.0330x over previous
"""PointNetLK on 8 TRN2 NeuronCores — batch-parallel, 2 samples/core.

prog1: 7 PointNet feature evals (tf + 6 finite-diff Jacobian evals), fp32r.
prog2: 10 LK iterations on-device: feat eval, pose via precomputed -pinv
       (sign-mapped into Se3-hat "seg" layout by host), SE3 exp as the
       matrix polynomial G = I + S + B*S^2 + C*S^3 on 8x8 blockdiag tiles.
Host: means, J assembly, 6x6 solve, final 4x4 assembly.

Layout: 2 samples/core stacked. Points in homogeneous form: ts8 [8,1024]
rows 0-2 = sample-a points^T, row 3 = ones, rows 4-6 = sample-b, row 7 = ones.
L1 weights in [8,128] blocks carrying rotation-folded W1 + bias row.
"""

import numpy as np

B, N, NC, SPC = 16, 1024, 8, 2
MAXITER = 10
N_TTR = 11  # of 16 L5 maxpool pairs: this many via ACT-copy + fused ttr

_BUILT = {}
TRACE = False
LAST_NS = 0


def _exp_se3_np(x):
    x = np.asarray(x, np.float64)
    w, v = x[..., :3], x[..., 3:]
    t2 = (w * w).sum(-1)
    t = np.sqrt(np.maximum(t2, 1e-300))
    small = t2 < 1e-12
    A = np.where(small, 1.0 - t2 / 6.0, np.sin(t) / t)
    Bc = np.where(small, 0.5 - t2 / 24.0, (1.0 - np.cos(t)) / np.maximum(t2, 1e-300))
    C = np.where(small, 1.0 / 6.0 - t2 / 120.0, (t - np.sin(t)) / np.maximum(t2 * t, 1e-300))
    z = np.zeros_like(t2)
    wx, wy, wz = w[..., 0], w[..., 1], w[..., 2]
    W = np.stack([
        np.stack([z, -wz, wy], -1),
        np.stack([wz, z, -wx], -1),
        np.stack([-wy, wx, z], -1)], -2)
    W2 = W @ W
    I = np.eye(3)
    R = I + A[..., None, None] * W + Bc[..., None, None] * W2
    V = I + Bc[..., None, None] * W + C[..., None, None] * W2
    tv = np.einsum('...ij,...j->...i', V, v)
    out = np.zeros(x.shape[:-1] + (4, 4))
    out[..., :3, :3] = R
    out[..., :3, 3] = tv
    out[..., 3, 3] = 1.0
    return out


def _feat_eval(nc, bigps, pairps, ts8, l18_ap,
               w2, w3, w4a, w4b, w5, x1, x2, x3, x4a, x4b, fdst):
    import concourse.mybir as mybir
    Relu = mybir.ActivationFunctionType.Relu
    mx = mybir.AluOpType.max
    H = 512

    def mm_act(lhsT, rhs_tile, out_tile):
        for h in range(2):
            p = bigps()
            nc.tensor.matmul(p[:, 0:H], lhsT, rhs_tile[:, h * H:(h + 1) * H],
                             start=True, stop=True)
            nc.scalar.activation(out_tile[:, h * H:(h + 1) * H], p[:, 0:H],
                                 Relu)

    mm_act(l18_ap, ts8, x1)
    mm_act(w2[:], x1, x2)
    mm_act(w3[:], x2, x3)
    mm_act(w4a[:], x3, x4a)
    mm_act(w4b[:], x3, x4b)
    for s, x4 in ((0, x4a), (1, x4b)):
        for j in range(8):
            pp = pairps()
            w5j = w5[:, 128 * j:128 * (j + 1)]
            nc.tensor.matmul(pp[:, 0:H], w5j, x4[:, 0:H],
                             start=True, stop=True)
            nc.tensor.matmul(pp[:, H:2 * H], w5j, x4[:, H:2 * H],
                             start=True, stop=True)
            col = 8 * s + j
            nc.vector.tensor_reduce(fdst[:, col:col + 1], pp[:],
                                    axis=mybir.AxisListType.X, op=mx)
    # clamp at zero (relu after max over all points)
    nc.vector.tensor_scalar(out=fdst[:], in0=fdst[:], scalar1=0.0,
                            scalar2=None, op0=mx)


def _build_common(nc, sb, dt_):
    ts8 = sb.tile([8, 1024], dt_)
    w2 = sb.tile([128, 128], dt_)
    w3 = sb.tile([128, 128], dt_)
    w4a = sb.tile([128, 128], dt_)
    w4b = sb.tile([128, 128], dt_)
    w5 = sb.tile([128, 1024], dt_)
    x1 = sb.tile([128, 1024], dt_)
    x2 = sb.tile([128, 1024], dt_)
    x3 = sb.tile([128, 1024], dt_)
    x4a = sb.tile([128, 1024], dt_)
    x4b = sb.tile([128, 1024], dt_)
    return ts8, w2, w3, w4a, w4b, w5, x1, x2, x3, x4a, x4b


def _make_pools(nc, tc):
    import concourse.mybir as mybir
    F32 = mybir.dt.float32
    ctxs = dict(
        sb=tc.tile_pool(name="sb", bufs=1),
        scr=tc.tile_pool(name="scr", bufs=3),
        junk=tc.tile_pool(name="junk", bufs=2),
        psb=tc.tile_pool(name="psb", bufs=2, space="PSUM"),
        psp=tc.tile_pool(name="psp", bufs=2, space="PSUM"),
        pss=tc.tile_pool(name="pss", bufs=2, space="PSUM"),
    )
    return ctxs


def _build_prog1(n_evals=7):
    import concourse.bacc as bacc
    import concourse.mybir as mybir
    import concourse.tile as tile
    F32 = mybir.dt.float32
    F32R = mybir.dt.float32r
    nc = bacc.Bacc()
    d = {}
    for name, shp in (("TS8", [8, 1024]), ("L1T8", [8, 896]),
                      ("W2B", [128, 128]), ("W3B", [128, 128]),
                      ("W4A", [128, 128]), ("W4B", [128, 128]),
                      ("W5", [128, 1024])):
        d[name] = nc.declare_dram_parameter(name, shp, F32R, isOutput=False)
    F7 = nc.declare_dram_parameter("F7", [128, 112], F32, isOutput=True)

    with tile.TileContext(nc) as tc:
        with (tc.tile_pool(name="sb", bufs=1) as sb,
              tc.tile_pool(name="psb", bufs=2, space="PSUM") as psb,
              tc.tile_pool(name="psp", bufs=3, space="PSUM") as psp):
            ts8, w2, w3, w4a, w4b, w5, x1, x2, x3, x4a, x4b = _build_common(nc, sb, F32R)
            l1t = sb.tile([8, 896], F32R)
            feats = sb.tile([128, 112], F32)
            for t_, d_ in ((ts8, d["TS8"]), (l1t, d["L1T8"]),
                           (w2, d["W2B"]), (w3, d["W3B"]), (w4a, d["W4A"]),
                           (w4b, d["W4B"]), (w5, d["W5"])):
                nc.sync.dma_start(t_[:], d_[:])

            def bigps():
                return psb.tile([128, 512], F32, name="bp", tag="bp")

            def pairps():
                return psp.tile([128, 1024], F32, name="pp", tag="pp")

            for e in range(n_evals):
                _feat_eval(nc, bigps, pairps, ts8,
                           l1t[:, 128 * e:128 * e + 128],
                           w2, w3, w4a, w4b, w5, x1, x2, x3, x4a, x4b,
                           feats[:, 16 * e:16 * e + 16])
            nc.sync.dma_start(F7[:], feats[:])
    nc.finalize()
    return nc


def _build_prog2():
    import concourse.bacc as bacc
    import concourse.mybir as mybir
    import concourse.tile as tile
    F32 = mybir.dt.float32
    F32R = mybir.dt.float32r
    mul = mybir.AluOpType.mult
    add = mybir.AluOpType.add
    Copy = mybir.ActivationFunctionType.Copy
    nc = bacc.Bacc()
    d = {}
    for name, shp in (("W1BLK8", [8, 128]),
                      ("CSEG", [1, 32]), ("SEL2", [2, 8]), ("EYE8", [8, 8]),
                      ("ONE11", [1, 1]), ("SEL32", [32, 8]),
                      ("MASK32", [32, 8])):
        d[name] = nc.declare_dram_parameter(name, shp, F32, isOutput=False)
    d["PVX"] = nc.declare_dram_parameter("PVX", [128, 256],
                                         mybir.dt.bfloat16, isOutput=False)
    BF16 = mybir.dt.bfloat16
    for name, shp in (("TS8", [8, 1024]),
                      ("W2B", [128, 128]), ("W3B", [128, 128]),
                      ("W4A", [128, 128]), ("W4B", [128, 128]),
                      ("W5", [128, 1024])):
        d[name] = nc.declare_dram_parameter(name, shp, BF16, isOutput=False)
    O = nc.declare_dram_parameter("O", [8, 8], F32, isOutput=True)

    with tile.TileContext(nc) as tc:
        with (tc.tile_pool(name="sb", bufs=1) as sb,
              tc.tile_pool(name="psb", bufs=2, space="PSUM") as psb,
              tc.tile_pool(name="psp", bufs=2, space="PSUM") as psp,
              tc.tile_pool(name="pss", bufs=2, space="PSUM") as pss):
            ts8, w2, w3, w4a, w4b, w5, x1, x2, x3, x4a, x4b = _build_common(nc, sb, BF16)
            w1blk = sb.tile([8, 128], F32)
            pvx = sb.tile([128, 256], BF16)
            cseg = sb.tile([1, 32], F32)
            sel2 = sb.tile([2, 8], F32)
            eye8 = sb.tile([8, 8], F32)
            one11 = sb.tile([1, 1], F32)
            sel32 = sb.tile([32, 8], F32)
            mask32 = sb.tile([32, 8], F32)
            l18 = sb.tile([8, 128], BF16)
            feats = sb.tile([128, 16], BF16)
            segSB = sb.tile([1, 32], F32)
            segcol = sb.tile([32, 1], F32)
            segm = sb.tile([32, 8], F32)
            sq6 = sb.tile([1, 6], F32)
            t2row = sb.tile([1, 2], F32)
            t2col = sb.tile([2, 1], F32)
            bc22 = sb.tile([2, 2], F32)
            s8 = sb.tile([8, 8], F32)
            st8 = sb.tile([8, 8], F32)
            s2t = sb.tile([8, 8], F32)
            gt1 = sb.tile([8, 8], F32)
            gt2 = sb.tile([8, 8], F32)
            gts = sb.tile([8, 8], F32)
            tsb = [sb.tile([8, 8], F32, name="tsb0"),
                   sb.tile([8, 8], F32, name="tsb1")]

            for t_, d_ in ((ts8, d["TS8"]), (w1blk, d["W1BLK8"]),
                           (pvx, d["PVX"]), (cseg, d["CSEG"]),
                           (sel2, d["SEL2"]), (eye8, d["EYE8"]),
                           (one11, d["ONE11"]), (sel32, d["SEL32"]),
                           (mask32, d["MASK32"]),
                           (w2, d["W2B"]), (w3, d["W3B"]), (w4a, d["W4A"]),
                           (w4b, d["W4B"]), (w5, d["W5"])):
                nc.sync.dma_start(t_[:], d_[:])

            nc.vector.tensor_copy(tsb[0][:], eye8[:])

            def bigps():
                return psb.tile([128, 512], F32, name="bp", tag="bp")

            def pairps():
                return psp.tile([128, 1024], F32, name="pp", tag="pp")

            def sps(shape):
                return pss.tile(shape, F32, name="sp", tag="sp")

            for it in range(MAXITER):
                Tcur = tsb[it % 2]
                Tnext = tsb[(it + 1) % 2]
                # fold est_T into L1 block: l18 = Tcur^T @ W1BLK8
                pf = sps([8, 128])
                nc.tensor.matmul(pf[:, 0:128], Tcur[:], w1blk[:],
                                 start=True, stop=True)
                nc.scalar.activation(l18[:], pf[:, 0:128], Copy)

                _feat_eval(nc, bigps, pairps, ts8, l18[:],
                           w2, w3, w4a, w4b, w5, x1, x2, x3, x4a, x4b,
                           feats[:])

                # pose in "seg" layout [1,32]: CSEG + sum_j PVX_chunk^T feats
                psg = sps([1, 32])
                for s in range(SPC):
                    sl = psg[0:1, 16 * s:16 * s + 16]
                    nc.tensor.matmul(sl, one11[:],
                                     cseg[0:1, 16 * s:16 * s + 16],
                                     start=True, stop=False,
                                     skip_group_check=True)
                    for j in range(8):
                        q = 8 * s + j
                        nc.tensor.matmul(sl, feats[:, q:q + 1],
                                         pvx[:, 16 * q:16 * q + 16],
                                         start=False, stop=(j == 7),
                                         skip_group_check=True)
                # S-hat assembly, engine-only: seg row -> column -> masked
                # broadcast -> select-matmul scatters into the 8x8 blockdiag
                nc.vector.tensor_copy(segSB[:], psg[0:1, 0:32])
                pcol = sps([32, 1])
                nc.tensor.matmul(pcol[0:32, 0:1], segSB[:], one11[:],
                                 start=True, stop=True)
                nc.vector.tensor_copy(segcol[:], pcol[0:32, 0:1])
                nc.vector.tensor_scalar(out=segm[:], in0=mask32[:],
                                        scalar1=segcol[:], scalar2=None,
                                        op0=mul)
                ps8 = sps([8, 8])
                nc.tensor.matmul(ps8[0:8, 0:8], sel32[:], segm[:],
                                 start=True, stop=True)
                nc.vector.tensor_copy(s8[:], ps8[0:8, 0:8])
                pst = sps([8, 8])
                nc.tensor.transpose(pst[0:8, 0:8], s8[:], eye8[:])
                nc.vector.tensor_copy(st8[:], pst[0:8, 0:8])

                # t2 = |w|^2 per sample from seg extras (slots 12-14, 28-30)
                nc.scalar.square(
                    sq6[:].rearrange("p (a c) -> p a c", a=2),
                    psg[0:1, 0:32].rearrange("p (a c) -> p a c", a=2, c=16)[:, :, 12:15])
                nc.vector.tensor_reduce(
                    t2row[:], sq6[:].rearrange("p (a c) -> p a c", a=2),
                    axis=mybir.AxisListType.X, op=add)
                pt2 = sps([2, 1])
                nc.tensor.matmul(pt2[0:2, 0:1], t2row[:], one11[:],
                                 start=True, stop=True)
                nc.vector.tensor_copy(t2col[:], pt2[0:2, 0:1])
                # Horner for B (col 0) and C (col 1) on [2,1]
                for col, (c3, c2, c1, c0) in (
                        (0, (-1.0 / 40320, 1.0 / 720, -1.0 / 24, 0.5)),
                        (1, (-1.0 / 362880, 1.0 / 5040, -1.0 / 120, 1.0 / 6))):
                    dst = bc22[0:2, col:col + 1]
                    nc.vector.tensor_scalar(out=dst, in0=t2col[:],
                                            scalar1=c3, scalar2=c2,
                                            op0=mul, op1=add)
                    nc.vector.tensor_scalar(out=dst, in0=dst,
                                            scalar1=t2col[:], scalar2=c1,
                                            op0=mul, op1=add)
                    nc.vector.tensor_scalar(out=dst, in0=dst,
                                            scalar1=t2col[:], scalar2=c0,
                                            op0=mul, op1=add)
                pbc = sps([8, 2])
                nc.tensor.matmul(pbc[0:8, 0:2], sel2[:], bc22[:],
                                 start=True, stop=True)

                # (S^2)^T and (S^3)^T
                ps2 = sps([8, 8])
                nc.tensor.matmul(ps2[0:8, 0:8], s8[:], st8[:],
                                 start=True, stop=True)
                nc.vector.tensor_copy(s2t[:], ps2[0:8, 0:8])
                ps3 = sps([8, 8])
                nc.tensor.matmul(ps3[0:8, 0:8], s8[:], s2t[:],
                                 start=True, stop=True)
                # G^T = I + S^T + B (S^2)^T + C (S^3)^T
                nc.vector.scalar_tensor_tensor(
                    out=gt1[:], in0=ps2[0:8, 0:8], scalar=pbc[0:8, 0:1],
                    in1=st8[:], op0=mul, op1=add)
                nc.vector.scalar_tensor_tensor(
                    out=gt2[:], in0=ps3[0:8, 0:8], scalar=pbc[0:8, 1:2],
                    in1=eye8[:], op0=mul, op1=add)
                nc.vector.tensor_tensor(out=gts[:], in0=gt1[:], in1=gt2[:],
                                        op=add)
                # T_next = G @ T_cur
                pT = sps([8, 8])
                nc.tensor.matmul(pT[0:8, 0:8], gts[:], Tcur[:],
                                 start=True, stop=True)
                nc.vector.tensor_copy(Tnext[:], pT[0:8, 0:8])

            nc.sync.dma_start(O[:], tsb[MAXITER % 2][:])
    nc.finalize()
    return nc


def _get_progs():
    if "p1" not in _BUILT:
        _BUILT["p1"] = _build_prog1()
        _BUILT["p2"] = _build_prog2()
    return _BUILT["p1"], _BUILT["p2"]


# seg slot -> (pose component k, sign); slots 0,5,10,15 are zero
_SEG_MAP = {1: (2, -1.0), 2: (1, 1.0), 3: (3, 1.0),
            4: (2, 1.0), 6: (0, -1.0), 7: (4, 1.0),
            8: (1, -1.0), 9: (0, 1.0), 11: (5, 1.0),
            12: (0, 1.0), 13: (1, 1.0), 14: (2, 1.0)}


def kernel(template, source, W1, b1, W2, b2, W3, b3, W4, b4, W5, b5, dt, maxiter):
    global LAST_NS
    from concourse.bass_utils import run_bass_kernel_spmd

    template = np.asarray(template, np.float32)
    source = np.asarray(source, np.float32)
    W1 = np.asarray(W1, np.float64)
    W2 = np.asarray(W2, np.float32)
    W3 = np.asarray(W3, np.float32)
    W4 = np.asarray(W4, np.float32)
    W5 = np.asarray(W5, np.float32)
    dtv = float(np.asarray(dt).reshape(-1)[0])

    m0 = template.mean(1)  # [B,3]
    m1 = source.mean(1)

    # shared weight blocks
    W2B = np.zeros((128, 128), np.float32)
    W2B[0:64, 0:64] = W2
    W2B[64:128, 64:128] = W2
    W3B = np.zeros((128, 128), np.float32)
    W3B[0:64, 0:64] = W3
    W3B[64:128, 64:128] = W3
    W4Az = np.zeros((128, 128), np.float32)
    W4Az[0:64, :] = W4
    W4Bz = np.zeros((128, 128), np.float32)
    W4Bz[64:128, :] = W4
    W5c = np.ascontiguousarray(W5)

    # J-eval transforms (host, constant given dt)
    twists = -np.eye(6) * dtv
    G = _exp_se3_np(twists)  # [6,4,4]
    Rs = [np.eye(3)] + [G[k, :3, :3] for k in range(6)]
    vs = [np.zeros(3)] + [G[k, :3, 3] for k in range(6)]

    p1, p2 = _get_progs()

    in_maps1 = []
    for c in range(NC):
        TS8 = np.zeros((8, 1024), np.float32)
        L1T8 = np.zeros((8, 896), np.float32)
        for s in range(SPC):
            b = SPC * c + s
            TS8[4 * s:4 * s + 3, :] = (template[b] - m0[b]).T
            TS8[4 * s + 3, :] = 1.0
            for e in range(7):
                lb = (Rs[e].T @ W1).astype(np.float32)
                L1T8[4 * s:4 * s + 3, 128 * e + 64 * s:128 * e + 64 * s + 64] = lb
                L1T8[4 * s + 3, 128 * e + 64 * s:128 * e + 64 * s + 64] = \
                    (W1.T @ vs[e]).astype(np.float32)
        in_maps1.append({"TS8": TS8, "L1T8": L1T8, "W2B": W2B,
                         "W3B": W3B, "W4A": W4Az, "W4B": W4Bz, "W5": W5c})

    r1 = run_bass_kernel_spmd(p1, in_maps1, list(range(NC)), trace=TRACE)
    ns1 = r1.exec_time_ns or 0

    # host: J, H, pinv, and seg-mapped PVX/CSEG
    PVXs, CSEGs = [], []
    for c in range(NC):
        F7 = r1.results[c]["F7"].astype(np.float64)  # [128,112]
        PVX = np.zeros((128, 256), np.float32)
        CSEG = np.zeros((1, 32), np.float32)
        for s in range(SPC):
            fe = np.zeros((7, 1024))
            for e in range(7):
                for j in range(8):
                    fe[e, 128 * j:128 * j + 128] = F7[:, 16 * e + 8 * s + j]
            tfv = fe[0]
            J = (tfv[:, None] - fe[1:7].T) / dtv  # [1024,6]
            Hm = J.T @ J
            pinv = np.linalg.solve(Hm, J.T)  # [6,1024]
            P = -pinv          # pose = P @ sf + cvec
            cvec = pinv @ tfv  # [6]
            for j in range(8):
                q = 8 * s + j
                Pj = P[:, 128 * j:128 * j + 128]  # [6,128]
                for slot, (k, sgn) in _SEG_MAP.items():
                    PVX[:, 16 * q + slot] = sgn * Pj[k]
            for slot, (k, sgn) in _SEG_MAP.items():
                CSEG[0, 16 * s + slot] = sgn * cvec[k]
        PVXs.append(PVX)
        CSEGs.append(CSEG)

    W1BLK8 = np.zeros((8, 128), np.float32)
    W1BLK8[0:3, 0:64] = W1.astype(np.float32)
    W1BLK8[4:7, 64:128] = W1.astype(np.float32)
    SEL2 = np.zeros((2, 8), np.float32)
    SEL2[0, 0:4] = 1.0
    SEL2[1, 4:8] = 1.0
    # seg slot c -> S-hat (row, col); select/mask consts for the scatter mm
    SEL32 = np.zeros((32, 8), np.float32)
    MASK32 = np.zeros((32, 8), np.float32)
    for cslot in range(32):
        s_, slot = cslot // 16, cslot % 16
        if slot >= 12 or slot in (0, 5, 10):
            continue
        SEL32[cslot, 4 * s_ + slot // 4] = 1.0
        MASK32[cslot, 4 * s_ + slot % 4] = 1.0

    import ml_dtypes
    bf = ml_dtypes.bfloat16
    in_maps2 = []
    for c in range(NC):
        TS8 = np.zeros((8, 1024), np.float32)
        for s in range(SPC):
            b = SPC * c + s
            TS8[4 * s:4 * s + 3, :] = (source[b] - m1[b]).T
            TS8[4 * s + 3, :] = 1.0
        in_maps2.append({"TS8": TS8.astype(bf), "W1BLK8": W1BLK8,
                         "PVX": PVXs[c].astype(bf),
                         "CSEG": CSEGs[c], "SEL2": SEL2,
                         "EYE8": np.eye(8, dtype=np.float32),
                         "ONE11": np.ones((1, 1), np.float32),
                         "SEL32": SEL32, "MASK32": MASK32,
                         "W2B": W2B.astype(bf), "W3B": W3B.astype(bf),
                         "W4A": W4Az.astype(bf), "W4B": W4Bz.astype(bf),
                         "W5": W5c.astype(bf)})

    r2 = run_bass_kernel_spmd(p2, in_maps2, list(range(NC)), trace=TRACE)
    ns2 = r2.exec_time_ns or 0
    LAST_NS = ns1 + ns2

    out = np.zeros((B, 4, 4), np.float32)
    for c in range(NC):
        O = r2.results[c]["O"]  # [8,8]
        for s in range(SPC):
            b = SPC * c + s
            R = O[4 * s:4 * s + 3, 4 * s:4 * s + 3].astype(np.float64)
            t = O[4 * s:4 * s + 3, 4 * s + 3].astype(np.float64)
            tfin = m0[b] + t - R @ m1[b]
            out[b, :3, :3] = R.astype(np.float32)
            out[b, :3, 3] = tfin.astype(np.float32)
            out[b, 3, 3] = 1.0
    return out
